# revision 2
# baseline (speedup 1.0000x reference)
"""Trainium2 Bass kernel for nn_HG_60481729462790 (gnn_message_passing).

Computes, for 800k (rna, dis) edge pairs over a shared embedding table:
    norm  = ||emb_row||_2 + 0.1 per row; emb_n = emb / norm
    logit = emb_n[rna_i] @ We @ emb_n[20000 + dis_j]^T   per edge
    returns (concat(pos_logits, neg_logits), concat(ones, zeros))

Strategy (8 NeuronCores, data-parallel over edges):
  - Each core builds two DRAM tables (replicated work, cheap):
      rna_n[20096,128] = normalized emb rows 0..20095
      dwn  [5000,128]  = We @ normalized dis row (normalization folded in)
    so that  logit(e) = dot(rna_n[i_e], dwn[j_e]).
  - Edge stream is sharded 100k/core. Per 2048-edge block: two indirect-DMA
    gathers (128 partitions x 16 rows each; one 512B descriptor per row),
    a DVE elementwise multiply and a segmented reduce -> 2048 logits.
  - Outputs are concatenated on host; label vector is a constant.
"""

import numpy as np

import concourse.bass as bass
import concourse.bacc as bacc
import concourse.mybir as mybir
import concourse.tile as tile
from concourse.bass import IndirectOffsetOnAxis
from concourse.masks import make_identity
from concourse.tile_rust import add_dep_helper

P = 128            # partitions
H = 128            # hidden
N_EMB = 25000
N_RNA = 20000
N_DIS = 5000
E_TOT = 800000
N_CORES = 8
E_CORE = E_TOT // N_CORES          # 100000 edges per core
K = 16                             # gathered rows per partition per block
BLK = P * K                        # 2048 edges per block
N_FULL = E_CORE // BLK             # 48 full blocks
TAIL = E_CORE - N_FULL * BLK       # 1696
TAIL_P = TAIL // K                 # 106 partitions in the tail block
GROUP = 8                          # blocks per output flush (16384 edges)

N_RNA_PAD = ((N_RNA + P - 1) // P) * P   # 20096 (rows 20000..20095 are
                                         # normalized dis rows; never indexed)

F32 = mybir.dt.float32
I32 = mybir.dt.int32

TABLE_DT = F32     # dtype of the gather tables (flip to bfloat16 to A/B)


ST = 8             # row-tiles per phase-A supertile (1024 rows per iteration)


def _norm_scale_wide(nc, pool, x_ap, p_used, tw):
    """Scale tile [P, tw] with 1/(||row||+0.1) for rows of x_ap [p_used, tw, H].

    Sum-of-squares runs on DVE so the ACT engine only ever executes Sqrt —
    mixing activation functions reloads the 1.3us activation table per op.
    """
    sq = pool.tile([P, ST, H], F32, tag="sq")
    sq2 = sq[:p_used, :tw, :].rearrange("p t h -> p (t h)")
    x2 = x_ap.rearrange("p t h -> p (t h)")
    nc.vector.tensor_tensor(out=sq2, in0=x2, in1=x2, op=mybir.AluOpType.mult)
    ss = pool.tile([P, ST], F32, tag="ss")
    nc.vector.reduce_sum(out=ss[:p_used, :tw], in_=sq[:p_used, :tw, :],
                         axis=mybir.AxisListType.X)
    nc.scalar.activation(
        out=ss[:p_used, :tw], in_=ss[:p_used, :tw],
        func=mybir.ActivationFunctionType.Sqrt,
    )
    nc.vector.tensor_scalar_add(ss[:p_used, :tw], ss[:p_used, :tw], 0.1)
    rec = pool.tile([P, ST], F32, tag="rec")
    nc.vector.reciprocal(rec[:p_used, :tw], ss[:p_used, :tw])
    return rec


def geom(k):
    blk = P * k
    n_full = E_CORE // blk
    tail = E_CORE - n_full * blk
    group = 128 // k          # blocks per score tile so GROUP*k == 128
    n_blocks = n_full + (1 if tail else 0)
    return blk, n_full, tail, group, n_blocks


def build_program(table_dt=TABLE_DT, phase_a=True, phase_b=True,
                  gathers=2, compute=True, flush=True, repeat_b=1,
                  k=K, single_packet=False):
    blk, n_full, tail, group, _nb = geom(k)
    nc = bacc.Bacc()

    emb = nc.dram_tensor("emb", [N_EMB, H], F32, kind="ExternalInput")
    we = nc.dram_tensor("We", [H, H], F32, kind="ExternalInput")
    # edge indices, host-prepared in dma_gather's wrapped-int16 layout:
    # plane column block b holds block b's 2048 indices with element i at
    # [i % 16, i // 16] (tail block padded with -1 = "skip")
    n_blocks = _nb
    ipw = blk // 16                       # index-plane columns per block
    ridx = nc.dram_tensor("rna_idx16", [16, n_blocks * ipw], mybir.dt.int16,
                          kind="ExternalInput")
    didx = nc.dram_tensor("dis_idx16", [16, n_blocks * ipw], mybir.dt.int16,
                          kind="ExternalInput")
    out = nc.dram_tensor("logits", [E_CORE], F32, kind="ExternalOutput")

    rna_n = nc.dram_tensor("rna_n", [N_RNA_PAD, H], table_dt, kind="Internal")
    dwn = nc.dram_tensor("dwn", [N_DIS, H], table_dt, kind="Internal")

    table_writes = []

    with tile.TileContext(nc) as tc:
        with (
            tc.tile_pool(name="const", bufs=1) as const_pool,
            tc.tile_pool(name="pa", bufs=3) as pa_pool,
            tc.tile_pool(name="pa_small", bufs=4) as pa_small,
            tc.tile_pool(name="pa_psum", bufs=2, space="PSUM") as pa_psum,
            tc.tile_pool(name="pb_idx", bufs=4) as pb_idx,
            tc.tile_pool(name="pb_gat", bufs=3) as pb_gat,
            tc.tile_pool(name="pb_sc", bufs=2) as pb_sc,
        ):
            # --- Phase A0: We^T in SBUF ------------------------------------
            # PE (Matmult) instructions tolerate only ONE sync-wait in walrus
            # codegen, so every PE input must be produced by the same engine
            # (DVE): route identity and We through DVE copies.
            ident0 = const_pool.tile([P, P], F32)
            make_identity(nc, ident0[:])
            ident = const_pool.tile([P, P], F32)
            nc.vector.tensor_copy(out=ident[:], in_=ident0[:])
            we_s0 = const_pool.tile([P, H], F32)
            nc.sync.dma_start(out=we_s0[:], in_=we[:, :])
            we_s = const_pool.tile([P, H], F32)
            nc.vector.tensor_copy(out=we_s[:], in_=we_s0[:])
            wet_ps = pa_psum.tile([P, P], F32, tag="wet")
            nc.tensor.transpose(out=wet_ps[:], in_=we_s[:], identity=ident[:])
            wet = const_pool.tile([P, H], F32)   # wet[h, ho] = We[ho, h]
            nc.vector.tensor_copy(out=wet[:], in_=wet_ps[:])

            # --- Phase A1: normalized rna table (rows 0..20095) ------------
            # Supertiles of ST row-tiles: wide DVE ops amortize the fixed
            # per-instruction overhead that dominated the per-tile version.
            n_st = (N_RNA_PAD // P + ST - 1) // ST if phase_a else 0
            for st in range(n_st):
                r0 = st * ST * P
                tw = min(ST, (N_RNA_PAD - r0) // P)
                x = pa_pool.tile([P, ST, H], F32, tag="ax")
                nc.sync.dma_start(
                    out=x[:, :tw, :],
                    in_=emb[r0:r0 + tw * P, :].rearrange("(t p) h -> p t h", p=P))
                rec = _norm_scale_wide(nc, pa_small, x[:, :tw, :], P, tw)
                y = pa_pool.tile([P, ST, H], table_dt, tag="ay")
                nc.vector.tensor_tensor(
                    out=y[:, :tw, :], in0=x[:, :tw, :],
                    in1=rec[:, :tw].to_broadcast([P, tw, H]),
                    op=mybir.AluOpType.mult)
                w = nc.sync.dma_start(
                    out=rna_n[r0:r0 + tw * P, :].rearrange("(t p) h -> p t h", p=P),
                    in_=y[:, :tw, :])
                table_writes.append(w)

            # --- Phase A2: dwn table (We @ normalized dis rows) ------------
            # norm on supertiles; transpose+matmul per 128-row subtile
            N_DIS_FULL = (N_DIS // P) * P               # 4992
            n_dst = (N_DIS_FULL // P + ST - 1) // ST if phase_a else 0
            for st in range(n_dst):
                d0 = st * ST * P
                tw = min(ST, (N_DIS_FULL - d0) // P)
                x = pa_pool.tile([P, ST, H], F32, tag="dx")
                nc.sync.dma_start(
                    out=x[:, :tw, :],
                    in_=emb[N_RNA + d0:N_RNA + d0 + tw * P, :].rearrange(
                        "(t p) h -> p t h", p=P))
                rec = _norm_scale_wide(nc, pa_small, x[:, :tw, :], P, tw)
                dn = pa_pool.tile([P, ST, H], F32, tag="dn")
                nc.vector.tensor_tensor(
                    out=dn[:, :tw, :], in0=x[:, :tw, :],
                    in1=rec[:, :tw].to_broadcast([P, tw, H]),
                    op=mybir.AluOpType.mult)
                for t in range(tw):
                    dnt_ps = pa_psum.tile([P, P], F32, tag="dnt")
                    nc.tensor.transpose(out=dnt_ps[:], in_=dn[:, t, :],
                                        identity=ident[:])
                    dnt = pa_pool.tile([P, P], F32, tag="dnts")
                    nc.vector.tensor_copy(out=dnt[:], in_=dnt_ps[:])
                    # out[d, ho] = sum_h dn[d,h] * We[ho,h]
                    mm_ps = pa_psum.tile([P, H], F32, tag="mm")
                    nc.tensor.matmul(out=mm_ps[:], lhsT=dnt[:],
                                     rhs=wet[:], start=True, stop=True)
                    z = pa_pool.tile([P, H], table_dt, tag="az")
                    nc.vector.tensor_copy(out=z[:], in_=mm_ps[:])
                    w = nc.sync.dma_start(
                        out=dwn[d0 + t * P:d0 + (t + 1) * P, :], in_=z[:])
                    table_writes.append(w)

            # ragged dis tail (rows 4992..4999)
            for _ in range(1 if phase_a else 0):
                rows = N_DIS - N_DIS_FULL                # 8
                d0 = N_DIS_FULL
                x = pa_pool.tile([P, ST, H], F32, tag="dx")
                nc.sync.dma_start(
                    out=x[:rows, 0, :],
                    in_=emb[N_RNA + d0:N_RNA + d0 + rows, :])
                rec = _norm_scale_wide(nc, pa_small, x[:rows, :1, :], rows, 1)
                dn = pa_pool.tile([P, ST, H], F32, tag="dn")
                nc.vector.tensor_tensor(
                    out=dn[:rows, :1, :], in0=x[:rows, :1, :],
                    in1=rec[:rows, :1].to_broadcast([rows, 1, H]),
                    op=mybir.AluOpType.mult)
                dnt_ps = pa_psum.tile([P, P], F32, tag="dnt")
                nc.tensor.transpose(out=dnt_ps[:, :rows], in_=dn[:rows, 0, :],
                                    identity=ident[:rows, :rows])
                dnt = pa_pool.tile([P, P], F32, tag="dnts")
                nc.vector.tensor_copy(out=dnt[:, :rows], in_=dnt_ps[:, :rows])
                mm_ps = pa_psum.tile([P, H], F32, tag="mm")
                nc.tensor.matmul(out=mm_ps[:rows, :], lhsT=dnt[:, :rows],
                                 rhs=wet[:], start=True, stop=True)
                z = pa_pool.tile([P, H], table_dt, tag="az")
                nc.vector.tensor_copy(out=z[:rows], in_=mm_ps[:rows, :])
                w = nc.sync.dma_start(out=dwn[d0:d0 + rows, :], in_=z[:rows])
                table_writes.append(w)

            # Fence: every gather must observe the completed tables.
            fence = nc.gpsimd.nop(nofuse=True, hint="table_fence")
            for w in table_writes:
                add_dep_helper(fence.ins, w.ins, reason="fence waits on table writes")

            # --- Phase B: gather + dot per 2048-edge block -----------------
            # dma_gather places row i at dest[p=i%128, slot=i//128], so edge
            # q (block-local) = s*128 + p lands at score[p, s].  A group of 8
            # blocks gives score_grp[128, 128] with col c = b_loc*16 + s; its
            # PE transpose has row c = 128 contiguous output edges.
            # Load both wrapped index planes once, replicated 8x down the
            # partition axis (one 16-partition copy per pair of Q7 cores).
            ridx_s = const_pool.tile([P, n_blocks * ipw], mybir.dt.int16)
            didx_s = const_pool.tile([P, n_blocks * ipw], mybir.dt.int16)
            for c in range(8):
                nc.sync.dma_start(out=ridx_s[16 * c:16 * (c + 1), :], in_=ridx[:, :])
                nc.sync.dma_start(out=didx_s[16 * c:16 * (c + 1), :], in_=didx[:, :])

            score = None
            for _rep_b in range((n_blocks if phase_b else 0) * repeat_b):
                b = _rep_b % max(n_blocks, 1)
                full = b < n_full
                n_valid = blk if full else tail

                g = b % group
                if g == 0:
                    score = pb_sc.tile([P, group * k], F32, tag="score")
                    if n_blocks - b < group:
                        # partial final group: zero unused columns so the
                        # full-tile transpose reads defined data
                        nc.vector.memset(score[:, :], 0.0)

                r = pb_gat.tile([P, k, H], table_dt, tag="r")
                d = pb_gat.tile([P, k, H], table_dt, tag="d")
                if not full:
                    # tail: gather skips the -1-padded rows; zero the tiles so
                    # the full-tile multiply/reduce reads defined data
                    nc.vector.memset(r[:, :, :], 0.0)
                    nc.vector.memset(d[:, :, :], 0.0)
                g1 = nc.gpsimd.dma_gather(
                    r[:, :, :], rna_n[:, :], ridx_s[:, b * ipw:(b + 1) * ipw],
                    num_idxs=blk, num_idxs_reg=n_valid,
                    elem_size=H, elem_step=H, single_packet=single_packet)
                add_dep_helper(g1.ins, fence.ins, reason="gather after tables")
                if gathers >= 2:
                    g2 = nc.gpsimd.dma_gather(
                        d[:, :, :], dwn[:, :], didx_s[:, b * ipw:(b + 1) * ipw],
                        num_idxs=blk, num_idxs_reg=n_valid,
                        elem_size=H, elem_step=H, single_packet=single_packet)
                    add_dep_helper(g2.ins, fence.ins, reason="gather after tables")
                elif compute:
                    nc.vector.memset(d[:, :, :], 0.5)

                if compute:
                    r2 = r[:, :, :].rearrange("p s h -> p (s h)")
                    d2 = d[:, :, :].rearrange("p s h -> p (s h)")
                    nc.vector.tensor_mul(r2, r2, d2)
                    nc.vector.reduce_sum(
                        out=score[:, g * k:(g + 1) * k], in_=r[:, :, :],
                        axis=mybir.AxisListType.X)

                # flush finished group via PE transpose -> contiguous DMA
                last_in_group = (g == group - 1) or (b == n_blocks - 1)
                if last_in_group and compute and flush:
                    e_g0 = (b - g) * blk
                    st_ps = pa_psum.tile([P, P], F32, tag="st")
                    nc.tensor.transpose(out=st_ps[:], in_=score[:, :],
                                        identity=ident[:])
                    st = pb_sc.tile([P, P], F32, tag="st_s")
                    nc.vector.tensor_copy(out=st[:], in_=st_ps[:])
                    n_out = (g * blk) + n_valid          # edges in this group
                    rows, rem = divmod(n_out, P)
                    if rows:
                        nc.sync.dma_start(
                            out=out[e_g0:e_g0 + rows * P].rearrange(
                                "(c p) -> c p", p=P),
                            in_=st[:rows, :])
                    if rem:
                        nc.sync.dma_start(
                            out=out[e_g0 + rows * P:e_g0 + n_out].rearrange(
                                "(o e) -> o e", o=1),
                            in_=st[rows:rows + 1, :rem])

    # Bacc pipeline: splits multi-waits into event semaphores (walrus allows
    # only one sync-wait per instruction), register alloc, DCE, etc.
    nc.compile()
    return nc


_PROGRAM_CACHE = {}


def _get_program(table_dt=TABLE_DT):
    key = str(table_dt)
    if key not in _PROGRAM_CACHE:
        _PROGRAM_CACHE[key] = build_program(table_dt)
    return _PROGRAM_CACHE[key]


def wrap_indices(idx, k=K):
    """[E_CORE] int -> dma_gather wrapped plane [16, n_blocks*k] int16.

    Block b's blk indices (tail padded with -1) occupy plane columns
    [b*k, (b+1)*k) with element i at [i % 16, i // 16].
    """
    blk, n_full, tail, group, n_blocks = geom(k)
    padded = np.full(n_blocks * blk, -1, dtype=np.int16)
    padded[:len(idx)] = idx.astype(np.int16)
    blocks = padded.reshape(n_blocks, blk // 16, 16).transpose(0, 2, 1)
    return np.ascontiguousarray(
        blocks.transpose(1, 0, 2).reshape(16, n_blocks * (blk // 16)))


def _make_in_maps(emb, We, rna_all, dis_all, k=K):
    in_maps = []
    for c in range(N_CORES):
        sl = slice(c * E_CORE, (c + 1) * E_CORE)
        in_maps.append({
            "emb": np.ascontiguousarray(emb, dtype=np.float32),
            "We": np.ascontiguousarray(We, dtype=np.float32),
            "rna_idx16": wrap_indices(np.asarray(rna_all[sl]), k),
            "dis_idx16": wrap_indices(np.asarray(dis_all[sl]), k),
        })
    return in_maps


def kernel_run(emb, We, pos_rna, pos_dis, neg_rna, neg_dis, rna_num,
               trace=False, table_dt=TABLE_DT):
    """Returns ((logits, label), exec_time_ns_or_None)."""
    from concourse.bass_utils import run_bass_kernel_spmd

    emb = np.asarray(emb)
    We = np.asarray(We)
    rna_all = np.concatenate([np.asarray(pos_rna), np.asarray(neg_rna)])
    dis_all = np.concatenate([np.asarray(pos_dis), np.asarray(neg_dis)])
    assert emb.shape == (N_EMB, H) and We.shape == (H, H)
    assert rna_all.shape == (E_TOT,) and dis_all.shape == (E_TOT,)

    nc = _get_program(table_dt)
    in_maps = _make_in_maps(emb, We, rna_all, dis_all)
    res = run_bass_kernel_spmd(
        nc, in_maps, core_ids=list(range(N_CORES)), trace=trace)
    global _LAST_RES
    _LAST_RES = res

    logits = np.concatenate([res.results[c]["logits"] for c in range(N_CORES)])
    n_pos = np.asarray(pos_rna).shape[0]
    n_neg = np.asarray(neg_rna).shape[0]
    label = np.concatenate([np.ones(n_pos, np.float32),
                            np.zeros(n_neg, np.float32)])
    return (logits.astype(np.float32), label), res.exec_time_ns


def kernel(**inputs):
    (logits, label), _ = kernel_run(**inputs)
    return (logits, label)



# revision 5
# speedup vs baseline: 1.9298x; 1.9298x over previous
"""Trainium2 Bass kernel for nn_HG_60481729462790 (gnn_message_passing).

Computes, for 800k (rna, dis) edge pairs over a shared embedding table:
    norm  = ||emb_row||_2 + 0.1 per row; emb_n = emb / norm
    logit = emb_n[rna_i] @ We @ emb_n[20000 + dis_j]^T   per edge
    returns (concat(pos_logits, neg_logits), concat(ones, zeros))

Strategy (8 NeuronCores, data-parallel over edges):
  - Each core builds two DRAM tables (replicated work, cheap):
      rna_n[20096,128] = normalized emb rows 0..20095
      dwn  [5000,128]  = We @ normalized dis row (normalization folded in)
    so that  logit(e) = dot(rna_n[i_e], dwn[j_e]).
  - Edge stream is sharded 100k/core. Per 2048-edge block: two indirect-DMA
    gathers (128 partitions x 16 rows each; one 512B descriptor per row),
    a DVE elementwise multiply and a segmented reduce -> 2048 logits.
  - Outputs are concatenated on host; label vector is a constant.
"""

import numpy as np

import concourse.bass as bass
import concourse.bacc as bacc
import concourse.mybir as mybir
import concourse.tile as tile
from concourse.bass import IndirectOffsetOnAxis
from concourse.masks import make_identity
from concourse.tile_rust import add_dep_helper

P = 128            # partitions
H = 128            # hidden
N_EMB = 25000
N_RNA = 20000
N_DIS = 5000
E_TOT = 800000
N_CORES = 8
E_CORE = E_TOT // N_CORES          # 100000 edges per core
K = 16                             # gathered rows per partition per block
BLK = P * K                        # 2048 edges per block
N_FULL = E_CORE // BLK             # 48 full blocks
TAIL = E_CORE - N_FULL * BLK       # 1696
TAIL_P = TAIL // K                 # 106 partitions in the tail block
GROUP = 8                          # blocks per output flush (16384 edges)

N_RNA_PAD = ((N_RNA + P - 1) // P) * P   # 20096 (rows 20000..20095 are
                                         # normalized dis rows; never indexed)

F32 = mybir.dt.float32
I32 = mybir.dt.int32

TABLE_DT = F32     # dtype of the gather tables (flip to bfloat16 to A/B)


ST = 8             # row-tiles per phase-A supertile (1024 rows per iteration)


def _norm_scale_wide(nc, pool, x_ap, p_used, tw):
    """Scale tile [P, tw] with 1/(||row||+0.1) for rows of x_ap [p_used, tw, H].

    Sum-of-squares runs on DVE so the ACT engine only ever executes Sqrt —
    mixing activation functions reloads the 1.3us activation table per op.
    """
    sq = pool.tile([P, ST, H], F32, tag="sq")
    sq2 = sq[:p_used, :tw, :].rearrange("p t h -> p (t h)")
    x2 = x_ap.rearrange("p t h -> p (t h)")
    nc.vector.tensor_tensor(out=sq2, in0=x2, in1=x2, op=mybir.AluOpType.mult)
    ss = pool.tile([P, ST], F32, tag="ss")
    nc.vector.reduce_sum(out=ss[:p_used, :tw], in_=sq[:p_used, :tw, :],
                         axis=mybir.AxisListType.X)
    nc.scalar.activation(
        out=ss[:p_used, :tw], in_=ss[:p_used, :tw],
        func=mybir.ActivationFunctionType.Sqrt,
    )
    nc.vector.tensor_scalar_add(ss[:p_used, :tw], ss[:p_used, :tw], 0.1)
    rec = pool.tile([P, ST], F32, tag="rec")
    nc.vector.reciprocal(rec[:p_used, :tw], ss[:p_used, :tw])
    return rec


def geom(k):
    blk = P * k
    n_full = E_CORE // blk
    tail = E_CORE - n_full * blk
    group = 128 // k          # blocks per score tile so GROUP*k == 128
    n_blocks = n_full + (1 if tail else 0)
    return blk, n_full, tail, group, n_blocks


def build_program(table_dt=TABLE_DT, phase_a=True, phase_b=True,
                  gathers=2, compute=True, flush=True, repeat_b=1,
                  k=K, single_packet=False, queue_rr=True):
    blk, n_full, tail, group, _nb = geom(k)
    nc = bacc.Bacc(num_swdge_queues=4 if queue_rr else 1)

    emb = nc.dram_tensor("emb", [N_EMB, H], F32, kind="ExternalInput")
    we = nc.dram_tensor("We", [H, H], F32, kind="ExternalInput")
    # edge indices, host-prepared in dma_gather's wrapped-int16 layout:
    # plane column block b holds block b's 2048 indices with element i at
    # [i % 16, i // 16] (tail block padded with -1 = "skip")
    n_blocks = _nb
    ipw = blk // 16                       # index-plane columns per block
    ridx = nc.dram_tensor("rna_idx16", [16, n_blocks * ipw], mybir.dt.int16,
                          kind="ExternalInput")
    didx = nc.dram_tensor("dis_idx16", [16, n_blocks * ipw], mybir.dt.int16,
                          kind="ExternalInput")
    out = nc.dram_tensor("logits", [E_CORE], F32, kind="ExternalOutput")

    rna_n = nc.dram_tensor("rna_n", [N_RNA_PAD, H], table_dt, kind="Internal")
    dwn = nc.dram_tensor("dwn", [N_DIS, H], table_dt, kind="Internal")

    table_writes = []

    with tile.TileContext(nc) as tc:
        with (
            tc.tile_pool(name="const", bufs=1) as const_pool,
            tc.tile_pool(name="pa", bufs=3) as pa_pool,
            tc.tile_pool(name="pa_small", bufs=4) as pa_small,
            tc.tile_pool(name="pa_psum", bufs=2, space="PSUM") as pa_psum,
            tc.tile_pool(name="pb_idx", bufs=4) as pb_idx,
            tc.tile_pool(name="pb_gat", bufs=3) as pb_gat,
            tc.tile_pool(name="pb_sc", bufs=2) as pb_sc,
        ):
            # --- Phase A0: We^T in SBUF ------------------------------------
            # PE (Matmult) instructions tolerate only ONE sync-wait in walrus
            # codegen, so every PE input must be produced by the same engine
            # (DVE): route identity and We through DVE copies.
            ident0 = const_pool.tile([P, P], F32)
            make_identity(nc, ident0[:])
            ident = const_pool.tile([P, P], F32)
            nc.vector.tensor_copy(out=ident[:], in_=ident0[:])
            we_s0 = const_pool.tile([P, H], F32)
            nc.sync.dma_start(out=we_s0[:], in_=we[:, :])
            we_s = const_pool.tile([P, H], F32)
            nc.vector.tensor_copy(out=we_s[:], in_=we_s0[:])
            wet_ps = pa_psum.tile([P, P], F32, tag="wet")
            nc.tensor.transpose(out=wet_ps[:], in_=we_s[:], identity=ident[:])
            wet = const_pool.tile([P, H], F32)   # wet[h, ho] = We[ho, h]
            nc.vector.tensor_copy(out=wet[:], in_=wet_ps[:])

            # --- Phase A1: normalized rna table (rows 0..20095) ------------
            # Supertiles of ST row-tiles: wide DVE ops amortize the fixed
            # per-instruction overhead that dominated the per-tile version.
            n_st = (N_RNA_PAD // P + ST - 1) // ST if phase_a else 0
            for st in range(n_st):
                r0 = st * ST * P
                tw = min(ST, (N_RNA_PAD - r0) // P)
                x = pa_pool.tile([P, ST, H], F32, tag="ax")
                nc.sync.dma_start(
                    out=x[:, :tw, :],
                    in_=emb[r0:r0 + tw * P, :].rearrange("(t p) h -> p t h", p=P))
                rec = _norm_scale_wide(nc, pa_small, x[:, :tw, :], P, tw)
                y = pa_pool.tile([P, ST, H], table_dt, tag="ay")
                nc.vector.tensor_tensor(
                    out=y[:, :tw, :], in0=x[:, :tw, :],
                    in1=rec[:, :tw].to_broadcast([P, tw, H]),
                    op=mybir.AluOpType.mult)
                w = nc.sync.dma_start(
                    out=rna_n[r0:r0 + tw * P, :].rearrange("(t p) h -> p t h", p=P),
                    in_=y[:, :tw, :])
                table_writes.append(w)

            # --- Phase A2: dwn table (We @ normalized dis rows) ------------
            # norm on supertiles; transpose+matmul per 128-row subtile
            N_DIS_FULL = (N_DIS // P) * P               # 4992
            n_dst = (N_DIS_FULL // P + ST - 1) // ST if phase_a else 0
            for st in range(n_dst):
                d0 = st * ST * P
                tw = min(ST, (N_DIS_FULL - d0) // P)
                x = pa_pool.tile([P, ST, H], F32, tag="dx")
                nc.sync.dma_start(
                    out=x[:, :tw, :],
                    in_=emb[N_RNA + d0:N_RNA + d0 + tw * P, :].rearrange(
                        "(t p) h -> p t h", p=P))
                rec = _norm_scale_wide(nc, pa_small, x[:, :tw, :], P, tw)
                dn = pa_pool.tile([P, ST, H], F32, tag="dn")
                nc.vector.tensor_tensor(
                    out=dn[:, :tw, :], in0=x[:, :tw, :],
                    in1=rec[:, :tw].to_broadcast([P, tw, H]),
                    op=mybir.AluOpType.mult)
                for t in range(tw):
                    dnt_ps = pa_psum.tile([P, P], F32, tag="dnt")
                    nc.tensor.transpose(out=dnt_ps[:], in_=dn[:, t, :],
                                        identity=ident[:])
                    dnt = pa_pool.tile([P, P], F32, tag="dnts")
                    nc.vector.tensor_copy(out=dnt[:], in_=dnt_ps[:])
                    # out[d, ho] = sum_h dn[d,h] * We[ho,h]
                    mm_ps = pa_psum.tile([P, H], F32, tag="mm")
                    nc.tensor.matmul(out=mm_ps[:], lhsT=dnt[:],
                                     rhs=wet[:], start=True, stop=True)
                    z = pa_pool.tile([P, H], table_dt, tag="az")
                    nc.vector.tensor_copy(out=z[:], in_=mm_ps[:])
                    w = nc.sync.dma_start(
                        out=dwn[d0 + t * P:d0 + (t + 1) * P, :], in_=z[:])
                    table_writes.append(w)

            # ragged dis tail (rows 4992..4999)
            for _ in range(1 if phase_a else 0):
                rows = N_DIS - N_DIS_FULL                # 8
                d0 = N_DIS_FULL
                x = pa_pool.tile([P, ST, H], F32, tag="dx")
                nc.sync.dma_start(
                    out=x[:rows, 0, :],
                    in_=emb[N_RNA + d0:N_RNA + d0 + rows, :])
                rec = _norm_scale_wide(nc, pa_small, x[:rows, :1, :], rows, 1)
                dn = pa_pool.tile([P, ST, H], F32, tag="dn")
                nc.vector.tensor_tensor(
                    out=dn[:rows, :1, :], in0=x[:rows, :1, :],
                    in1=rec[:rows, :1].to_broadcast([rows, 1, H]),
                    op=mybir.AluOpType.mult)
                dnt_ps = pa_psum.tile([P, P], F32, tag="dnt")
                nc.tensor.transpose(out=dnt_ps[:, :rows], in_=dn[:rows, 0, :],
                                    identity=ident[:rows, :rows])
                dnt = pa_pool.tile([P, P], F32, tag="dnts")
                nc.vector.tensor_copy(out=dnt[:, :rows], in_=dnt_ps[:, :rows])
                mm_ps = pa_psum.tile([P, H], F32, tag="mm")
                nc.tensor.matmul(out=mm_ps[:rows, :], lhsT=dnt[:, :rows],
                                 rhs=wet[:], start=True, stop=True)
                z = pa_pool.tile([P, H], table_dt, tag="az")
                nc.vector.tensor_copy(out=z[:rows], in_=mm_ps[:rows, :])
                w = nc.sync.dma_start(out=dwn[d0:d0 + rows, :], in_=z[:rows])
                table_writes.append(w)

            # Fence: every gather must observe the completed tables.
            fence = nc.gpsimd.nop(nofuse=True, hint="table_fence")
            for w in table_writes:
                add_dep_helper(fence.ins, w.ins, reason="fence waits on table writes")

            # --- Phase B: gather + dot per 2048-edge block -----------------
            # dma_gather places row i at dest[p=i%128, slot=i//128], so edge
            # q (block-local) = s*128 + p lands at score[p, s].  A group of 8
            # blocks gives score_grp[128, 128] with col c = b_loc*16 + s; its
            # PE transpose has row c = 128 contiguous output edges.
            # Load both wrapped index planes once, replicated 8x down the
            # partition axis (one 16-partition copy per pair of Q7 cores).
            ridx_s = const_pool.tile([P, n_blocks * ipw], mybir.dt.int16)
            didx_s = const_pool.tile([P, n_blocks * ipw], mybir.dt.int16)
            for c in range(8):
                nc.sync.dma_start(out=ridx_s[16 * c:16 * (c + 1), :], in_=ridx[:, :])
                nc.sync.dma_start(out=didx_s[16 * c:16 * (c + 1), :], in_=didx[:, :])

            score = None
            for _rep_b in range((n_blocks if phase_b else 0) * repeat_b):
                b = _rep_b % max(n_blocks, 1)
                full = b < n_full
                n_valid = blk if full else tail

                g = b % group
                if g == 0:
                    score = pb_sc.tile([P, group * k], F32, tag="score")
                    if n_blocks - b < group:
                        # partial final group: zero unused columns so the
                        # full-tile transpose reads defined data
                        nc.vector.memset(score[:, :], 0.0)

                r = pb_gat.tile([P, k, H], table_dt, tag="r")
                d = pb_gat.tile([P, k, H], table_dt, tag="d")
                if not full:
                    # tail: gather skips the -1-padded rows; zero the tiles so
                    # the full-tile multiply/reduce reads defined data
                    nc.vector.memset(r[:, :, :], 0.0)
                    nc.vector.memset(d[:, :, :], 0.0)
                # dma_gather's Q7 ucode runs only on core pair `queue_num`
                # (cpu_id/2 == queue_num); rotating queues lets up to 4
                # gathers generate descriptors concurrently.
                q1 = (2 * b) % 4 if queue_rr else 0
                q2 = (2 * b + 1) % 4 if queue_rr else 0
                g1 = nc.gpsimd.dma_gather(
                    r[:, :, :], rna_n[:, :], ridx_s[:, b * ipw:(b + 1) * ipw],
                    num_idxs=blk, num_idxs_reg=n_valid,
                    elem_size=H, elem_step=H, single_packet=single_packet,
                    queue_num=q1)
                add_dep_helper(g1.ins, fence.ins, reason="gather after tables")
                if gathers >= 2:
                    g2 = nc.gpsimd.dma_gather(
                        d[:, :, :], dwn[:, :], didx_s[:, b * ipw:(b + 1) * ipw],
                        num_idxs=blk, num_idxs_reg=n_valid,
                        elem_size=H, elem_step=H, single_packet=single_packet,
                        queue_num=q2)
                    add_dep_helper(g2.ins, fence.ins, reason="gather after tables")
                elif compute:
                    nc.vector.memset(d[:, :, :], 0.5)

                if compute:
                    r2 = r[:, :, :].rearrange("p s h -> p (s h)")
                    d2 = d[:, :, :].rearrange("p s h -> p (s h)")
                    nc.vector.tensor_mul(r2, r2, d2)
                    nc.vector.reduce_sum(
                        out=score[:, g * k:(g + 1) * k], in_=r[:, :, :],
                        axis=mybir.AxisListType.X)

                # flush finished group via PE transpose -> contiguous DMA
                last_in_group = (g == group - 1) or (b == n_blocks - 1)
                if last_in_group and compute and flush:
                    e_g0 = (b - g) * blk
                    st_ps = pa_psum.tile([P, P], F32, tag="st")
                    nc.tensor.transpose(out=st_ps[:], in_=score[:, :],
                                        identity=ident[:])
                    st = pb_sc.tile([P, P], F32, tag="st_s")
                    nc.vector.tensor_copy(out=st[:], in_=st_ps[:])
                    n_out = (g * blk) + n_valid          # edges in this group
                    rows, rem = divmod(n_out, P)
                    if rows:
                        nc.sync.dma_start(
                            out=out[e_g0:e_g0 + rows * P].rearrange(
                                "(c p) -> c p", p=P),
                            in_=st[:rows, :])
                    if rem:
                        nc.sync.dma_start(
                            out=out[e_g0 + rows * P:e_g0 + n_out].rearrange(
                                "(o e) -> o e", o=1),
                            in_=st[rows:rows + 1, :rem])

    # Bacc pipeline: splits multi-waits into event semaphores (walrus allows
    # only one sync-wait per instruction), register alloc, DCE, etc.
    nc.compile()
    return nc


_PROGRAM_CACHE = {}


def _get_program(table_dt=TABLE_DT):
    key = str(table_dt)
    if key not in _PROGRAM_CACHE:
        _PROGRAM_CACHE[key] = build_program(table_dt)
    return _PROGRAM_CACHE[key]


def wrap_indices(idx, k=K):
    """[E_CORE] int -> dma_gather wrapped plane [16, n_blocks*k] int16.

    Block b's blk indices (tail padded with -1) occupy plane columns
    [b*k, (b+1)*k) with element i at [i % 16, i // 16].
    """
    blk, n_full, tail, group, n_blocks = geom(k)
    padded = np.full(n_blocks * blk, -1, dtype=np.int16)
    padded[:len(idx)] = idx.astype(np.int16)
    blocks = padded.reshape(n_blocks, blk // 16, 16).transpose(0, 2, 1)
    return np.ascontiguousarray(
        blocks.transpose(1, 0, 2).reshape(16, n_blocks * (blk // 16)))


def _make_in_maps(emb, We, rna_all, dis_all, k=K):
    in_maps = []
    for c in range(N_CORES):
        sl = slice(c * E_CORE, (c + 1) * E_CORE)
        in_maps.append({
            "emb": np.ascontiguousarray(emb, dtype=np.float32),
            "We": np.ascontiguousarray(We, dtype=np.float32),
            "rna_idx16": wrap_indices(np.asarray(rna_all[sl]), k),
            "dis_idx16": wrap_indices(np.asarray(dis_all[sl]), k),
        })
    return in_maps


def kernel_run(emb, We, pos_rna, pos_dis, neg_rna, neg_dis, rna_num,
               trace=False, table_dt=TABLE_DT):
    """Returns ((logits, label), exec_time_ns_or_None)."""
    from concourse.bass_utils import run_bass_kernel_spmd

    emb = np.asarray(emb)
    We = np.asarray(We)
    rna_all = np.concatenate([np.asarray(pos_rna), np.asarray(neg_rna)])
    dis_all = np.concatenate([np.asarray(pos_dis), np.asarray(neg_dis)])
    assert emb.shape == (N_EMB, H) and We.shape == (H, H)
    assert rna_all.shape == (E_TOT,) and dis_all.shape == (E_TOT,)

    nc = _get_program(table_dt)
    in_maps = _make_in_maps(emb, We, rna_all, dis_all)
    res = run_bass_kernel_spmd(
        nc, in_maps, core_ids=list(range(N_CORES)), trace=trace)
    global _LAST_RES
    _LAST_RES = res

    logits = np.concatenate([res.results[c]["logits"] for c in range(N_CORES)])
    n_pos = np.asarray(pos_rna).shape[0]
    n_neg = np.asarray(neg_rna).shape[0]
    label = np.concatenate([np.ones(n_pos, np.float32),
                            np.zeros(n_neg, np.float32)])
    return (logits.astype(np.float32), label), res.exec_time_ns


def kernel(**inputs):
    (logits, label), _ = kernel_run(**inputs)
    return (logits, label)



# revision 9
# speedup vs baseline: 2.6752x; 1.3863x over previous
"""Trainium2 Bass kernel for nn_HG_60481729462790 (gnn_message_passing).

Computes, for 800k (rna, dis) edge pairs over a shared embedding table:
    norm  = ||emb_row||_2 + 0.1 per row; emb_n = emb / norm
    logit = emb_n[rna_i] @ We @ emb_n[20000 + dis_j]^T   per edge
    returns (concat(pos_logits, neg_logits), concat(ones, zeros))

Strategy (8 NeuronCores, data-parallel over edges):
  - Each core builds two DRAM tables (replicated work, cheap):
      rna_n[20096,128] = normalized emb rows 0..20095
      dwn  [5000,128]  = We @ normalized dis row (normalization folded in)
    so that  logit(e) = dot(rna_n[i_e], dwn[j_e]).
  - Edge stream is sharded 100k/core. Per 2048-edge block: two indirect-DMA
    gathers (128 partitions x 16 rows each; one 512B descriptor per row),
    a DVE elementwise multiply and a segmented reduce -> 2048 logits.
  - Outputs are concatenated on host; label vector is a constant.
"""

import numpy as np

import concourse.bass as bass
import concourse.bacc as bacc
import concourse.mybir as mybir
import concourse.tile as tile
from concourse.bass import IndirectOffsetOnAxis
from concourse.masks import make_identity
from concourse.tile_rust import add_dep_helper

P = 128            # partitions
H = 128            # hidden
N_EMB = 25000
N_RNA = 20000
N_DIS = 5000
E_TOT = 800000
N_CORES = 8
E_CORE = E_TOT // N_CORES          # 100000 edges per core
K = 16                             # gathered rows per partition per block
BLK = P * K                        # 2048 edges per block
N_FULL = E_CORE // BLK             # 48 full blocks
TAIL = E_CORE - N_FULL * BLK       # 1696
TAIL_P = TAIL // K                 # 106 partitions in the tail block
GROUP = 8                          # blocks per output flush (16384 edges)

N_RNA_PAD = ((N_RNA + P - 1) // P) * P   # 20096 (rows 20000..20095 are
                                         # normalized dis rows; never indexed)

F32 = mybir.dt.float32
I32 = mybir.dt.int32
BF16 = mybir.dt.bfloat16

TABLE_DT = F32     # dtype of the gather tables (flip to bfloat16 to A/B)


ST = 8             # row-tiles per phase-A supertile (1024 rows per iteration)

# ---- v2 (sort-by-dis) geometry -------------------------------------------
N_BLK2 = (E_CORE + BLK - 1) // BLK          # 49 gather blocks
E_PAD = N_BLK2 * BLK                        # 100352 (352 pad edges)
T_TILES = E_PAD // P                        # 784 PE tiles of 128 edges
TB = 4                                      # tiles per DVE batch (1 PSUM bank)
SC_GROUP = 128                              # tiles per score-flush group
LJ_SENTINEL = 255                           # lj value that never matches iota


def _v2_windows():
    """Static 128-wide dwn windows per tile: edges are host-sorted by dis, so
    tile t's dis values concentrate at the t-th quantile (sigma ~8 rows; the
    +/-64 window is an ~8-sigma box).  Data-independent program constants."""
    ws = []
    for t in range(T_TILES):
        c = round((128 * t + 64) * N_DIS / E_CORE)
        ws.append(min(max(c - 64, 0), N_DIS - P))
    return ws


V2_WINDOWS = _v2_windows()


def _norm_scale_wide(nc, pool, x_ap, p_used, tw):
    """Scale tile [P, tw] with 1/(||row||+0.1) for rows of x_ap [p_used, tw, H].

    Sum-of-squares runs on DVE so the ACT engine only ever executes Sqrt —
    mixing activation functions reloads the 1.3us activation table per op.
    """
    sq = pool.tile([P, ST, H], F32, tag="sq")
    sq2 = sq[:p_used, :tw, :].rearrange("p t h -> p (t h)")
    x2 = x_ap.rearrange("p t h -> p (t h)")
    nc.vector.tensor_tensor(out=sq2, in0=x2, in1=x2, op=mybir.AluOpType.mult)
    ss = pool.tile([P, ST], F32, tag="ss")
    nc.vector.reduce_sum(out=ss[:p_used, :tw], in_=sq[:p_used, :tw, :],
                         axis=mybir.AxisListType.X)
    nc.scalar.activation(
        out=ss[:p_used, :tw], in_=ss[:p_used, :tw],
        func=mybir.ActivationFunctionType.Sqrt,
    )
    nc.vector.tensor_scalar_add(ss[:p_used, :tw], ss[:p_used, :tw], 0.1)
    rec = pool.tile([P, ST], F32, tag="rec")
    nc.vector.reciprocal(rec[:p_used, :tw], ss[:p_used, :tw])
    return rec


def geom(k):
    blk = P * k
    n_full = E_CORE // blk
    tail = E_CORE - n_full * blk
    group = 128 // k          # blocks per score tile so GROUP*k == 128
    n_blocks = n_full + (1 if tail else 0)
    return blk, n_full, tail, group, n_blocks


def build_program(table_dt=TABLE_DT, phase_a=True, phase_b=True,
                  gathers=2, compute=True, flush=True, repeat_b=1,
                  k=K, single_packet=False, queue_rr=True):
    blk, n_full, tail, group, _nb = geom(k)
    nc = bacc.Bacc(num_swdge_queues=4 if queue_rr else 1)

    emb = nc.dram_tensor("emb", [N_EMB, H], F32, kind="ExternalInput")
    we = nc.dram_tensor("We", [H, H], F32, kind="ExternalInput")
    # edge indices, host-prepared in dma_gather's wrapped-int16 layout:
    # plane column block b holds block b's 2048 indices with element i at
    # [i % 16, i // 16] (tail block padded with -1 = "skip")
    n_blocks = _nb
    ipw = blk // 16                       # index-plane columns per block
    ridx = nc.dram_tensor("rna_idx16", [16, n_blocks * ipw], mybir.dt.int16,
                          kind="ExternalInput")
    didx = nc.dram_tensor("dis_idx16", [16, n_blocks * ipw], mybir.dt.int16,
                          kind="ExternalInput")
    out = nc.dram_tensor("logits", [E_CORE], F32, kind="ExternalOutput")

    rna_n = nc.dram_tensor("rna_n", [N_RNA_PAD, H], table_dt, kind="Internal")
    dwn = nc.dram_tensor("dwn", [N_DIS, H], table_dt, kind="Internal")

    table_writes = []

    with tile.TileContext(nc) as tc:
        with (
            tc.tile_pool(name="const", bufs=1) as const_pool,
            tc.tile_pool(name="pa", bufs=3) as pa_pool,
            tc.tile_pool(name="pa_small", bufs=4) as pa_small,
            tc.tile_pool(name="pa_psum", bufs=2, space="PSUM") as pa_psum,
            tc.tile_pool(name="pb_idx", bufs=4) as pb_idx,
            tc.tile_pool(name="pb_gat", bufs=6) as pb_gat,
            tc.tile_pool(name="pb_sc", bufs=4) as pb_sc,
        ):
            # --- Phase A0: We^T in SBUF ------------------------------------
            # PE (Matmult) instructions tolerate only ONE sync-wait in walrus
            # codegen, so every PE input must be produced by the same engine
            # (DVE): route identity and We through DVE copies.
            ident0 = const_pool.tile([P, P], F32)
            make_identity(nc, ident0[:])
            ident = const_pool.tile([P, P], F32)
            nc.vector.tensor_copy(out=ident[:], in_=ident0[:])
            we_s0 = const_pool.tile([P, H], F32)
            nc.sync.dma_start(out=we_s0[:], in_=we[:, :])
            we_s = const_pool.tile([P, H], F32)
            nc.vector.tensor_copy(out=we_s[:], in_=we_s0[:])
            wet_ps = pa_psum.tile([P, P], F32, tag="wet")
            nc.tensor.transpose(out=wet_ps[:], in_=we_s[:], identity=ident[:])
            wet = const_pool.tile([P, H], F32)   # wet[h, ho] = We[ho, h]
            nc.vector.tensor_copy(out=wet[:], in_=wet_ps[:])

            # --- Phase A1: normalized rna table (rows 0..20095) ------------
            # Supertiles of ST row-tiles: wide DVE ops amortize the fixed
            # per-instruction overhead that dominated the per-tile version.
            n_st = (N_RNA_PAD // P + ST - 1) // ST if phase_a else 0
            for st in range(n_st):
                r0 = st * ST * P
                tw = min(ST, (N_RNA_PAD - r0) // P)
                x = pa_pool.tile([P, ST, H], F32, tag="ax")
                nc.sync.dma_start(
                    out=x[:, :tw, :],
                    in_=emb[r0:r0 + tw * P, :].rearrange("(t p) h -> p t h", p=P))
                rec = _norm_scale_wide(nc, pa_small, x[:, :tw, :], P, tw)
                y = pa_pool.tile([P, ST, H], table_dt, tag="ay")
                nc.vector.tensor_tensor(
                    out=y[:, :tw, :], in0=x[:, :tw, :],
                    in1=rec[:, :tw].to_broadcast([P, tw, H]),
                    op=mybir.AluOpType.mult)
                w = nc.sync.dma_start(
                    out=rna_n[r0:r0 + tw * P, :].rearrange("(t p) h -> p t h", p=P),
                    in_=y[:, :tw, :])
                table_writes.append(w)

            # --- Phase A2: dwn table (We @ normalized dis rows) ------------
            # norm on supertiles; transpose+matmul per 128-row subtile
            N_DIS_FULL = (N_DIS // P) * P               # 4992
            n_dst = (N_DIS_FULL // P + ST - 1) // ST if phase_a else 0
            for st in range(n_dst):
                d0 = st * ST * P
                tw = min(ST, (N_DIS_FULL - d0) // P)
                x = pa_pool.tile([P, ST, H], F32, tag="dx")
                nc.sync.dma_start(
                    out=x[:, :tw, :],
                    in_=emb[N_RNA + d0:N_RNA + d0 + tw * P, :].rearrange(
                        "(t p) h -> p t h", p=P))
                rec = _norm_scale_wide(nc, pa_small, x[:, :tw, :], P, tw)
                dn = pa_pool.tile([P, ST, H], F32, tag="dn")
                nc.vector.tensor_tensor(
                    out=dn[:, :tw, :], in0=x[:, :tw, :],
                    in1=rec[:, :tw].to_broadcast([P, tw, H]),
                    op=mybir.AluOpType.mult)
                for t in range(tw):
                    dnt_ps = pa_psum.tile([P, P], F32, tag="dnt")
                    nc.tensor.transpose(out=dnt_ps[:], in_=dn[:, t, :],
                                        identity=ident[:])
                    dnt = pa_pool.tile([P, P], F32, tag="dnts")
                    nc.vector.tensor_copy(out=dnt[:], in_=dnt_ps[:])
                    # out[d, ho] = sum_h dn[d,h] * We[ho,h]
                    mm_ps = pa_psum.tile([P, H], F32, tag="mm")
                    nc.tensor.matmul(out=mm_ps[:], lhsT=dnt[:],
                                     rhs=wet[:], start=True, stop=True)
                    z = pa_pool.tile([P, H], table_dt, tag="az")
                    nc.vector.tensor_copy(out=z[:], in_=mm_ps[:])
                    w = nc.sync.dma_start(
                        out=dwn[d0 + t * P:d0 + (t + 1) * P, :], in_=z[:])
                    table_writes.append(w)

            # ragged dis tail (rows 4992..4999)
            for _ in range(1 if phase_a else 0):
                rows = N_DIS - N_DIS_FULL                # 8
                d0 = N_DIS_FULL
                x = pa_pool.tile([P, ST, H], F32, tag="dx")
                nc.sync.dma_start(
                    out=x[:rows, 0, :],
                    in_=emb[N_RNA + d0:N_RNA + d0 + rows, :])
                rec = _norm_scale_wide(nc, pa_small, x[:rows, :1, :], rows, 1)
                dn = pa_pool.tile([P, ST, H], F32, tag="dn")
                nc.vector.tensor_tensor(
                    out=dn[:rows, :1, :], in0=x[:rows, :1, :],
                    in1=rec[:rows, :1].to_broadcast([rows, 1, H]),
                    op=mybir.AluOpType.mult)
                dnt_ps = pa_psum.tile([P, P], F32, tag="dnt")
                nc.tensor.transpose(out=dnt_ps[:, :rows], in_=dn[:rows, 0, :],
                                    identity=ident[:rows, :rows])
                dnt = pa_pool.tile([P, P], F32, tag="dnts")
                nc.vector.tensor_copy(out=dnt[:, :rows], in_=dnt_ps[:, :rows])
                mm_ps = pa_psum.tile([P, H], F32, tag="mm")
                nc.tensor.matmul(out=mm_ps[:rows, :], lhsT=dnt[:, :rows],
                                 rhs=wet[:], start=True, stop=True)
                z = pa_pool.tile([P, H], table_dt, tag="az")
                nc.vector.tensor_copy(out=z[:rows], in_=mm_ps[:rows, :])
                w = nc.sync.dma_start(out=dwn[d0:d0 + rows, :], in_=z[:rows])
                table_writes.append(w)

            # Fence: every gather must observe the completed tables.
            fence = nc.gpsimd.nop(nofuse=True, hint="table_fence")
            for w in table_writes:
                add_dep_helper(fence.ins, w.ins, reason="fence waits on table writes")

            # --- Phase B: gather + dot per 2048-edge block -----------------
            # dma_gather places row i at dest[p=i%128, slot=i//128], so edge
            # q (block-local) = s*128 + p lands at score[p, s].  A group of 8
            # blocks gives score_grp[128, 128] with col c = b_loc*16 + s; its
            # PE transpose has row c = 128 contiguous output edges.
            # Load both wrapped index planes once, replicated 8x down the
            # partition axis (one 16-partition copy per pair of Q7 cores).
            ridx_s = const_pool.tile([P, n_blocks * ipw], mybir.dt.int16)
            didx_s = const_pool.tile([P, n_blocks * ipw], mybir.dt.int16)
            for c in range(8):
                nc.sync.dma_start(out=ridx_s[16 * c:16 * (c + 1), :], in_=ridx[:, :])
                nc.sync.dma_start(out=didx_s[16 * c:16 * (c + 1), :], in_=didx[:, :])

            score = None
            for _rep_b in range((n_blocks if phase_b else 0) * repeat_b):
                b = _rep_b % max(n_blocks, 1)
                full = b < n_full
                n_valid = blk if full else tail

                g = b % group
                if g == 0:
                    score = pb_sc.tile([P, group * k], F32, tag="score")
                    if n_blocks - b < group:
                        # partial final group: zero unused columns so the
                        # full-tile transpose reads defined data
                        nc.vector.memset(score[:, :], 0.0)

                r = pb_gat.tile([P, k, H], table_dt, tag="r")
                d = pb_gat.tile([P, k, H], table_dt, tag="d")
                if not full:
                    # tail: gather skips the -1-padded rows; zero the tiles so
                    # the full-tile multiply/reduce reads defined data
                    nc.vector.memset(r[:, :, :], 0.0)
                    nc.vector.memset(d[:, :, :], 0.0)
                # dma_gather's Q7 ucode runs only on core pair `queue_num`
                # (cpu_id/2 == queue_num); rotating queues lets up to 4
                # gathers generate descriptors concurrently.
                q1 = (2 * b) % 4 if queue_rr else 0
                q2 = (2 * b + 1) % 4 if queue_rr else 0
                g1 = nc.gpsimd.dma_gather(
                    r[:, :, :], rna_n[:, :], ridx_s[:, b * ipw:(b + 1) * ipw],
                    num_idxs=blk, num_idxs_reg=n_valid,
                    elem_size=H, elem_step=H, single_packet=single_packet,
                    queue_num=q1)
                add_dep_helper(g1.ins, fence.ins, reason="gather after tables")
                if gathers >= 2:
                    g2 = nc.gpsimd.dma_gather(
                        d[:, :, :], dwn[:, :], didx_s[:, b * ipw:(b + 1) * ipw],
                        num_idxs=blk, num_idxs_reg=n_valid,
                        elem_size=H, elem_step=H, single_packet=single_packet,
                        queue_num=q2)
                    add_dep_helper(g2.ins, fence.ins, reason="gather after tables")
                elif compute:
                    nc.vector.memset(d[:, :, :], 0.5)

                if compute:
                    r2 = r[:, :, :].rearrange("p s h -> p (s h)")
                    d2 = d[:, :, :].rearrange("p s h -> p (s h)")
                    nc.vector.tensor_mul(r2, r2, d2)
                    nc.vector.reduce_sum(
                        out=score[:, g * k:(g + 1) * k], in_=r[:, :, :],
                        axis=mybir.AxisListType.X)

                # flush finished group via PE transpose -> contiguous DMA
                last_in_group = (g == group - 1) or (b == n_blocks - 1)
                if last_in_group and compute and flush:
                    e_g0 = (b - g) * blk
                    st_ps = pa_psum.tile([P, P], F32, tag="st")
                    nc.tensor.transpose(out=st_ps[:], in_=score[:, :],
                                        identity=ident[:])
                    st = pb_sc.tile([P, P], F32, tag="st_s")
                    nc.vector.tensor_copy(out=st[:], in_=st_ps[:])
                    n_out = (g * blk) + n_valid          # edges in this group
                    rows, rem = divmod(n_out, P)
                    if rows:
                        nc.sync.dma_start(
                            out=out[e_g0:e_g0 + rows * P].rearrange(
                                "(c p) -> c p", p=P),
                            in_=st[:rows, :])
                    if rem:
                        nc.sync.dma_start(
                            out=out[e_g0 + rows * P:e_g0 + n_out].rearrange(
                                "(o e) -> o e", o=1),
                            in_=st[rows:rows + 1, :rem])

    # Bacc pipeline: splits multi-waits into event semaphores (walrus allows
    # only one sync-wait per instruction), register alloc, DCE, etc.
    nc.compile()
    return nc


def build_program_v2():
    """Sort-by-dis design: the dis side of every edge dot product is
    reconstructed on the PE from an SBUF-resident dwn^T table via a one-hot
    matmul against a static 128-wide window, so only the rna side is an
    indirect gather (bf16, transpose mode, 4-way SWDGE queue round-robin).

    Per 128-edge tile t (edges host-sorted by dis, logits unpermuted there):
        G[e, j]  = rna_n[i_e] . dwn[w_t + j]      (PE: lhsT=r_T, rhs=dwn_T win)
        logit[e] = sum_j G[e, j] * (j == lj_e)    (DVE: is_equal, mult, reduce)
    """
    nc = bacc.Bacc(num_swdge_queues=4)

    emb = nc.dram_tensor("emb", [N_EMB, H], F32, kind="ExternalInput")
    we = nc.dram_tensor("We", [H, H], F32, kind="ExternalInput")
    ipw = BLK // 16
    ridx = nc.dram_tensor("rna_idx16", [16, N_BLK2 * ipw], mybir.dt.int16,
                          kind="ExternalInput")
    lj_in = nc.dram_tensor("lj16", [P, T_TILES], BF16, kind="ExternalInput")
    out = nc.dram_tensor("logits", [E_PAD], F32, kind="ExternalOutput")

    rna_n = nc.dram_tensor("rna_n", [N_RNA_PAD, H], BF16, kind="Internal")

    table_writes = []

    with tile.TileContext(nc) as tc:
        with (
            tc.tile_pool(name="const", bufs=1) as const_pool,
            tc.tile_pool(name="pa", bufs=3) as pa_pool,
            tc.tile_pool(name="pa_small", bufs=4) as pa_small,
            tc.tile_pool(name="pa_psum", bufs=2, space="PSUM") as pa_psum,
            tc.tile_pool(name="gat", bufs=6) as gat_pool,
            tc.tile_pool(name="oh", bufs=4) as oh_pool,
            tc.tile_pool(name="pr", bufs=4) as pr_pool,
            tc.tile_pool(name="sc", bufs=2) as sc_pool,
            tc.tile_pool(name="gp", bufs=3, space="PSUM") as gp_pool,
        ):
            # --- We^T in SBUF (PE inputs routed through DVE copies) --------
            ident0 = const_pool.tile([P, P], F32)
            make_identity(nc, ident0[:])
            ident = const_pool.tile([P, P], F32)
            nc.vector.tensor_copy(out=ident[:], in_=ident0[:])
            we_s0 = const_pool.tile([P, H], F32)
            nc.sync.dma_start(out=we_s0[:], in_=we[:, :])
            we_s = const_pool.tile([P, H], F32)
            nc.vector.tensor_copy(out=we_s[:], in_=we_s0[:])
            wet_ps = pa_psum.tile([P, P], F32, tag="wet")
            nc.tensor.transpose(out=wet_ps[:], in_=we_s[:], identity=ident[:])
            wet = const_pool.tile([P, H], F32)   # wet[h, ho] = We[ho, h]
            nc.vector.tensor_copy(out=wet[:], in_=wet_ps[:])

            # --- A1: normalized rna table (bf16, DRAM) ---------------------
            n_st = (N_RNA_PAD // P + ST - 1) // ST
            for st in range(n_st):
                r0 = st * ST * P
                tw = min(ST, (N_RNA_PAD - r0) // P)
                x = pa_pool.tile([P, ST, H], F32, tag="ax")
                nc.sync.dma_start(
                    out=x[:, :tw, :],
                    in_=emb[r0:r0 + tw * P, :].rearrange("(t p) h -> p t h", p=P))
                rec = _norm_scale_wide(nc, pa_small, x[:, :tw, :], P, tw)
                y = pa_pool.tile([P, ST, H], BF16, tag="ay")
                nc.vector.tensor_tensor(
                    out=y[:, :tw, :], in0=x[:, :tw, :],
                    in1=rec[:, :tw].to_broadcast([P, tw, H]),
                    op=mybir.AluOpType.mult)
                w = nc.sync.dma_start(
                    out=rna_n[r0:r0 + tw * P, :].rearrange("(t p) h -> p t h", p=P),
                    in_=y[:, :tw, :])
                table_writes.append(w)

            # --- A2: dwn^T resident in SBUF [h, d] bf16 --------------------
            # dwn[d, ho] = sum_h We[ho, h] * dis_n[d, h]; transposed form
            # dwn_T[ho, d] = matmul(lhsT=wet[h, ho], rhs=dnt[h, d]).
            dwn_t = const_pool.tile([P, N_DIS], BF16)
            N_DIS_FULL = (N_DIS // P) * P               # 4992
            n_dst = (N_DIS_FULL // P + ST - 1) // ST
            for st in range(n_dst):
                d0 = st * ST * P
                tw = min(ST, (N_DIS_FULL - d0) // P)
                x = pa_pool.tile([P, ST, H], F32, tag="dx")
                nc.sync.dma_start(
                    out=x[:, :tw, :],
                    in_=emb[N_RNA + d0:N_RNA + d0 + tw * P, :].rearrange(
                        "(t p) h -> p t h", p=P))
                rec = _norm_scale_wide(nc, pa_small, x[:, :tw, :], P, tw)
                dn = pa_pool.tile([P, ST, H], F32, tag="dn")
                nc.vector.tensor_tensor(
                    out=dn[:, :tw, :], in0=x[:, :tw, :],
                    in1=rec[:, :tw].to_broadcast([P, tw, H]),
                    op=mybir.AluOpType.mult)
                for t in range(tw):
                    dnt_ps = pa_psum.tile([P, P], F32, tag="dnt")
                    nc.tensor.transpose(out=dnt_ps[:], in_=dn[:, t, :],
                                        identity=ident[:])
                    dnt = pa_pool.tile([P, P], F32, tag="dnts")
                    nc.vector.tensor_copy(out=dnt[:], in_=dnt_ps[:])
                    mm_ps = pa_psum.tile([P, P], F32, tag="mm")
                    nc.tensor.matmul(out=mm_ps[:], lhsT=wet[:],
                                     rhs=dnt[:], start=True, stop=True)
                    nc.vector.tensor_copy(
                        out=dwn_t[:, d0 + t * P:d0 + (t + 1) * P], in_=mm_ps[:])

            # dis tail rows 4992..4999
            rows = N_DIS - N_DIS_FULL                    # 8
            d0 = N_DIS_FULL
            x = pa_pool.tile([P, ST, H], F32, tag="dx")
            nc.sync.dma_start(
                out=x[:rows, 0, :], in_=emb[N_RNA + d0:N_RNA + d0 + rows, :])
            rec = _norm_scale_wide(nc, pa_small, x[:rows, :1, :], rows, 1)
            dn = pa_pool.tile([P, ST, H], F32, tag="dn")
            nc.vector.tensor_tensor(
                out=dn[:rows, :1, :], in0=x[:rows, :1, :],
                in1=rec[:rows, :1].to_broadcast([rows, 1, H]),
                op=mybir.AluOpType.mult)
            dnt_ps = pa_psum.tile([P, P], F32, tag="dnt")
            nc.tensor.transpose(out=dnt_ps[:, :rows], in_=dn[:rows, 0, :],
                                identity=ident[:rows, :rows])
            dnt = pa_pool.tile([P, P], F32, tag="dnts")
            nc.vector.tensor_copy(out=dnt[:, :rows], in_=dnt_ps[:, :rows])
            mm_ps = pa_psum.tile([P, P], F32, tag="mm")
            nc.tensor.matmul(out=mm_ps[:, :rows], lhsT=wet[:],
                             rhs=dnt[:, :rows], start=True, stop=True)
            nc.vector.tensor_copy(out=dwn_t[:, d0:d0 + rows],
                                  in_=mm_ps[:, :rows])

            # Fence: gathers must observe the completed rna_n table.
            fence = nc.gpsimd.nop(nofuse=True, hint="table_fence")
            for w in table_writes:
                add_dep_helper(fence.ins, w.ins, reason="fence waits on table writes")

            # --- Phase-B constants ----------------------------------------
            # wrapped rna index planes, one 16-partition copy per Q7 core
            ridx_s = const_pool.tile([P, N_BLK2 * ipw], mybir.dt.int16)
            for c in range(8):
                nc.sync.dma_start(out=ridx_s[16 * c:16 * (c + 1), :], in_=ridx[:, :])
            # window-local dis index per edge: lj_s[e, t]
            lj_s = const_pool.tile([P, T_TILES], BF16)
            nc.sync.dma_start(out=lj_s[:, :], in_=lj_in[:, :])
            # iota over the window axis, replicated TB wide
            io32 = const_pool.tile([P, TB, P], I32)
            nc.gpsimd.iota(io32[:, :, :], pattern=[[0, TB], [1, P]],
                           base=0, channel_multiplier=0)
            iob = const_pool.tile([P, TB, P], BF16)
            nc.vector.tensor_copy(out=iob[:, :, :], in_=io32[:, :, :])

            # --- Phase B: gather + windowed one-hot dot per tile -----------
            score = None
            for b in range(N_BLK2):
                rt = gat_pool.tile([P, 1, BLK], BF16, tag="rt")
                g = nc.gpsimd.dma_gather(
                    rt[:, :, :], rna_n[:, :], ridx_s[:, b * ipw:(b + 1) * ipw],
                    num_idxs=BLK, num_idxs_reg=BLK, elem_size=H,
                    transpose=True, queue_num=b % 4)
                add_dep_helper(g.ins, fence.ins, reason="gather after tables")

                for tb in range(BLK // P // TB):          # 4 batches of TB tiles
                    t0 = b * (BLK // P) + tb * TB
                    g_loc = t0 % SC_GROUP
                    if g_loc == 0:
                        score = sc_pool.tile([P, SC_GROUP], F32, tag="score")
                        if T_TILES - t0 < SC_GROUP:
                            nc.vector.memset(score[:, :], 0.0)
                    gp_t = gp_pool.tile([P, TB, P], F32, tag="gp")
                    for u in range(TB):
                        t = t0 + u
                        e0 = (tb * TB + u) * P
                        wt = V2_WINDOWS[t]
                        nc.tensor.matmul(
                            out=gp_t[:, u, :],
                            lhsT=rt[:, 0, e0:e0 + P],
                            rhs=dwn_t[:, wt:wt + P],
                            start=True, stop=True)
                    oh_t = oh_pool.tile([P, TB, P], BF16, tag="oh")
                    nc.vector.tensor_tensor(
                        out=oh_t[:, :, :], in0=iob[:, :, :],
                        in1=lj_s[:, t0:t0 + TB].to_broadcast([P, TB, P]),
                        op=mybir.AluOpType.is_equal)
                    pr_t = pr_pool.tile([P, TB, P], F32, tag="pr")
                    nc.vector.tensor_tensor(
                        out=pr_t[:, :, :], in0=gp_t[:, :, :], in1=oh_t[:, :, :],
                        op=mybir.AluOpType.mult)
                    nc.vector.reduce_sum(
                        out=score[:, g_loc:g_loc + TB], in_=pr_t[:, :, :],
                        axis=mybir.AxisListType.X)

                    # flush finished score group (128 tiles = 16384 edges)
                    t_next = t0 + TB
                    if t_next % SC_GROUP == 0 or t_next == T_TILES:
                        grp = t0 // SC_GROUP
                        n_rows = min(T_TILES - grp * SC_GROUP, SC_GROUP)
                        st_ps = pa_psum.tile([P, P], F32, tag="st")
                        nc.tensor.transpose(out=st_ps[:], in_=score[:, :],
                                            identity=ident[:])
                        stt = sc_pool.tile([P, P], F32, tag="st_s")
                        nc.vector.tensor_copy(out=stt[:], in_=st_ps[:])
                        e_g0 = grp * SC_GROUP * P
                        nc.sync.dma_start(
                            out=out[e_g0:e_g0 + n_rows * P].rearrange(
                                "(c p) -> c p", p=P),
                            in_=stt[:n_rows, :])

    nc.compile()
    return nc


_PROGRAM_CACHE = {}


def _get_program(table_dt=TABLE_DT):
    key = str(table_dt)
    if key not in _PROGRAM_CACHE:
        _PROGRAM_CACHE[key] = build_program(table_dt)
    return _PROGRAM_CACHE[key]


def _get_program_v2():
    if "v2" not in _PROGRAM_CACHE:
        _PROGRAM_CACHE["v2"] = build_program_v2()
    return _PROGRAM_CACHE["v2"]


def wrap_indices(idx, k=K):
    """[E_CORE] int -> dma_gather wrapped plane [16, n_blocks*k] int16.

    Block b's blk indices (tail padded with -1) occupy plane columns
    [b*k, (b+1)*k) with element i at [i % 16, i // 16].
    """
    blk, n_full, tail, group, n_blocks = geom(k)
    padded = np.full(n_blocks * blk, -1, dtype=np.int16)
    padded[:len(idx)] = idx.astype(np.int16)
    blocks = padded.reshape(n_blocks, blk // 16, 16).transpose(0, 2, 1)
    return np.ascontiguousarray(
        blocks.transpose(1, 0, 2).reshape(16, n_blocks * (blk // 16)))


def _make_in_maps(emb, We, rna_all, dis_all, k=K):
    in_maps = []
    for c in range(N_CORES):
        sl = slice(c * E_CORE, (c + 1) * E_CORE)
        in_maps.append({
            "emb": np.ascontiguousarray(emb, dtype=np.float32),
            "We": np.ascontiguousarray(We, dtype=np.float32),
            "rna_idx16": wrap_indices(np.asarray(rna_all[sl]), k),
            "dis_idx16": wrap_indices(np.asarray(dis_all[sl]), k),
        })
    return in_maps


def kernel_run(emb, We, pos_rna, pos_dis, neg_rna, neg_dis, rna_num,
               trace=False, table_dt=TABLE_DT):
    """Returns ((logits, label), exec_time_ns_or_None)."""
    from concourse.bass_utils import run_bass_kernel_spmd

    emb = np.asarray(emb)
    We = np.asarray(We)
    rna_all = np.concatenate([np.asarray(pos_rna), np.asarray(neg_rna)])
    dis_all = np.concatenate([np.asarray(pos_dis), np.asarray(neg_dis)])
    assert emb.shape == (N_EMB, H) and We.shape == (H, H)
    assert rna_all.shape == (E_TOT,) and dis_all.shape == (E_TOT,)

    nc = _get_program(table_dt)
    in_maps = _make_in_maps(emb, We, rna_all, dis_all)
    res = run_bass_kernel_spmd(
        nc, in_maps, core_ids=list(range(N_CORES)), trace=trace)
    global _LAST_RES
    _LAST_RES = res

    logits = np.concatenate([res.results[c]["logits"] for c in range(N_CORES)])
    n_pos = np.asarray(pos_rna).shape[0]
    n_neg = np.asarray(neg_rna).shape[0]
    label = np.concatenate([np.ones(n_pos, np.float32),
                            np.zeros(n_neg, np.float32)])
    return (logits.astype(np.float32), label), res.exec_time_ns


def kernel(**inputs):
    (logits, label), _ = kernel_run(**inputs)
    return (logits, label)



# revision 13
# speedup vs baseline: 2.6858x; 1.0040x over previous
"""Trainium2 Bass kernel for nn_HG_60481729462790 (gnn_message_passing).

Computes, for 800k (rna, dis) edge pairs over a shared embedding table:
    norm  = ||emb_row||_2 + 0.1 per row; emb_n = emb / norm
    logit = emb_n[rna_i] @ We @ emb_n[20000 + dis_j]^T   per edge
    returns (concat(pos_logits, neg_logits), concat(ones, zeros))

Strategy (8 NeuronCores, data-parallel over edges):
  - Each core builds two DRAM tables (replicated work, cheap):
      rna_n[20096,128] = normalized emb rows 0..20095
      dwn  [5000,128]  = We @ normalized dis row (normalization folded in)
    so that  logit(e) = dot(rna_n[i_e], dwn[j_e]).
  - Edge stream is sharded 100k/core. Per 2048-edge block: two indirect-DMA
    gathers (128 partitions x 16 rows each; one 512B descriptor per row),
    a DVE elementwise multiply and a segmented reduce -> 2048 logits.
  - Outputs are concatenated on host; label vector is a constant.
"""

import numpy as np

import concourse.bass as bass
import concourse.bacc as bacc
import concourse.mybir as mybir
import concourse.tile as tile
from concourse.bass import IndirectOffsetOnAxis
from concourse.masks import make_identity
from concourse.tile_rust import add_dep_helper

P = 128            # partitions
H = 128            # hidden
N_EMB = 25000
N_RNA = 20000
N_DIS = 5000
E_TOT = 800000
N_CORES = 8
E_CORE = E_TOT // N_CORES          # 100000 edges per core
K = 16                             # gathered rows per partition per block
BLK = P * K                        # 2048 edges per block
N_FULL = E_CORE // BLK             # 48 full blocks
TAIL = E_CORE - N_FULL * BLK       # 1696
TAIL_P = TAIL // K                 # 106 partitions in the tail block
GROUP = 8                          # blocks per output flush (16384 edges)

N_RNA_PAD = ((N_RNA + P - 1) // P) * P   # 20096 (rows 20000..20095 are
                                         # normalized dis rows; never indexed)

F32 = mybir.dt.float32
I32 = mybir.dt.int32
BF16 = mybir.dt.bfloat16

TABLE_DT = F32     # dtype of the gather tables (flip to bfloat16 to A/B)


ST = 8             # row-tiles per phase-A supertile (1024 rows per iteration)

# ---- v2 (sort-by-dis) geometry -------------------------------------------
N_BLK2 = (E_CORE + BLK - 1) // BLK          # 49 gather blocks
E_PAD = N_BLK2 * BLK                        # 100352 (352 pad edges)
T_TILES = E_PAD // P                        # 784 PE tiles of 128 edges
TB = 4                                      # tiles per DVE batch (1 PSUM bank)
SC_GROUP = 128                              # tiles per score-flush group
LJ_SENTINEL = 255                           # lj value that never matches iota


def _v2_windows():
    """Static 128-wide dwn windows per tile: edges are host-sorted by dis, so
    tile t's dis values concentrate at the t-th quantile (sigma ~8 rows; the
    +/-64 window is an ~8-sigma box).  Data-independent program constants."""
    ws = []
    for t in range(T_TILES):
        c = round((128 * t + 64) * N_DIS / E_CORE)
        ws.append(min(max(c - 64, 0), N_DIS - P))
    return ws


V2_WINDOWS = _v2_windows()


def _norm_scale_wide(nc, pool, x_ap, p_used, tw):
    """Scale tile [P, tw] with 1/(||row||+0.1) for rows of x_ap [p_used, tw, H].

    Sum-of-squares runs on DVE so the ACT engine only ever executes Sqrt —
    mixing activation functions reloads the 1.3us activation table per op.
    """
    sq = pool.tile([P, ST, H], F32, tag="sq")
    sq2 = sq[:p_used, :tw, :].rearrange("p t h -> p (t h)")
    x2 = x_ap.rearrange("p t h -> p (t h)")
    nc.vector.tensor_tensor(out=sq2, in0=x2, in1=x2, op=mybir.AluOpType.mult)
    ss = pool.tile([P, ST], F32, tag="ss")
    nc.vector.reduce_sum(out=ss[:p_used, :tw], in_=sq[:p_used, :tw, :],
                         axis=mybir.AxisListType.X)
    nc.scalar.activation(
        out=ss[:p_used, :tw], in_=ss[:p_used, :tw],
        func=mybir.ActivationFunctionType.Sqrt,
    )
    nc.vector.tensor_scalar_add(ss[:p_used, :tw], ss[:p_used, :tw], 0.1)
    rec = pool.tile([P, ST], F32, tag="rec")
    nc.vector.reciprocal(rec[:p_used, :tw], ss[:p_used, :tw])
    return rec


def geom(k):
    blk = P * k
    n_full = E_CORE // blk
    tail = E_CORE - n_full * blk
    group = 128 // k          # blocks per score tile so GROUP*k == 128
    n_blocks = n_full + (1 if tail else 0)
    return blk, n_full, tail, group, n_blocks


def build_program(table_dt=TABLE_DT, phase_a=True, phase_b=True,
                  gathers=2, compute=True, flush=True, repeat_b=1,
                  k=K, single_packet=False, queue_rr=True):
    blk, n_full, tail, group, _nb = geom(k)
    nc = bacc.Bacc(num_swdge_queues=4 if queue_rr else 1)

    emb = nc.dram_tensor("emb", [N_EMB, H], F32, kind="ExternalInput")
    we = nc.dram_tensor("We", [H, H], F32, kind="ExternalInput")
    # edge indices, host-prepared in dma_gather's wrapped-int16 layout:
    # plane column block b holds block b's 2048 indices with element i at
    # [i % 16, i // 16] (tail block padded with -1 = "skip")
    n_blocks = _nb
    ipw = blk // 16                       # index-plane columns per block
    ridx = nc.dram_tensor("rna_idx16", [16, n_blocks * ipw], mybir.dt.int16,
                          kind="ExternalInput")
    didx = nc.dram_tensor("dis_idx16", [16, n_blocks * ipw], mybir.dt.int16,
                          kind="ExternalInput")
    out = nc.dram_tensor("logits", [E_CORE], F32, kind="ExternalOutput")

    rna_n = nc.dram_tensor("rna_n", [N_RNA_PAD, H], table_dt, kind="Internal")
    dwn = nc.dram_tensor("dwn", [N_DIS, H], table_dt, kind="Internal")

    table_writes = []

    with tile.TileContext(nc) as tc:
        with (
            tc.tile_pool(name="const", bufs=1) as const_pool,
            tc.tile_pool(name="pa", bufs=3) as pa_pool,
            tc.tile_pool(name="pa_small", bufs=4) as pa_small,
            tc.tile_pool(name="pa_psum", bufs=2, space="PSUM") as pa_psum,
            tc.tile_pool(name="pb_idx", bufs=4) as pb_idx,
            tc.tile_pool(name="pb_gat", bufs=6) as pb_gat,
            tc.tile_pool(name="pb_sc", bufs=4) as pb_sc,
        ):
            # --- Phase A0: We^T in SBUF ------------------------------------
            # PE (Matmult) instructions tolerate only ONE sync-wait in walrus
            # codegen, so every PE input must be produced by the same engine
            # (DVE): route identity and We through DVE copies.
            ident0 = const_pool.tile([P, P], F32)
            make_identity(nc, ident0[:])
            ident = const_pool.tile([P, P], F32)
            nc.vector.tensor_copy(out=ident[:], in_=ident0[:])
            we_s0 = const_pool.tile([P, H], F32)
            nc.sync.dma_start(out=we_s0[:], in_=we[:, :])
            we_s = const_pool.tile([P, H], F32)
            nc.vector.tensor_copy(out=we_s[:], in_=we_s0[:])
            wet_ps = pa_psum.tile([P, P], F32, tag="wet")
            nc.tensor.transpose(out=wet_ps[:], in_=we_s[:], identity=ident[:])
            wet = const_pool.tile([P, H], F32)   # wet[h, ho] = We[ho, h]
            nc.vector.tensor_copy(out=wet[:], in_=wet_ps[:])

            # --- Phase A1: normalized rna table (rows 0..20095) ------------
            # Supertiles of ST row-tiles: wide DVE ops amortize the fixed
            # per-instruction overhead that dominated the per-tile version.
            n_st = (N_RNA_PAD // P + ST - 1) // ST if phase_a else 0
            for st in range(n_st):
                r0 = st * ST * P
                tw = min(ST, (N_RNA_PAD - r0) // P)
                x = pa_pool.tile([P, ST, H], F32, tag="ax")
                nc.sync.dma_start(
                    out=x[:, :tw, :],
                    in_=emb[r0:r0 + tw * P, :].rearrange("(t p) h -> p t h", p=P))
                rec = _norm_scale_wide(nc, pa_small, x[:, :tw, :], P, tw)
                y = pa_pool.tile([P, ST, H], table_dt, tag="ay")
                nc.vector.tensor_tensor(
                    out=y[:, :tw, :], in0=x[:, :tw, :],
                    in1=rec[:, :tw].to_broadcast([P, tw, H]),
                    op=mybir.AluOpType.mult)
                w = nc.sync.dma_start(
                    out=rna_n[r0:r0 + tw * P, :].rearrange("(t p) h -> p t h", p=P),
                    in_=y[:, :tw, :])
                table_writes.append(w)

            # --- Phase A2: dwn table (We @ normalized dis rows) ------------
            # norm on supertiles; transpose+matmul per 128-row subtile
            N_DIS_FULL = (N_DIS // P) * P               # 4992
            n_dst = (N_DIS_FULL // P + ST - 1) // ST if phase_a else 0
            for st in range(n_dst):
                d0 = st * ST * P
                tw = min(ST, (N_DIS_FULL - d0) // P)
                x = pa_pool.tile([P, ST, H], F32, tag="dx")
                nc.sync.dma_start(
                    out=x[:, :tw, :],
                    in_=emb[N_RNA + d0:N_RNA + d0 + tw * P, :].rearrange(
                        "(t p) h -> p t h", p=P))
                rec = _norm_scale_wide(nc, pa_small, x[:, :tw, :], P, tw)
                dn = pa_pool.tile([P, ST, H], F32, tag="dn")
                nc.vector.tensor_tensor(
                    out=dn[:, :tw, :], in0=x[:, :tw, :],
                    in1=rec[:, :tw].to_broadcast([P, tw, H]),
                    op=mybir.AluOpType.mult)
                for t in range(tw):
                    dnt_ps = pa_psum.tile([P, P], F32, tag="dnt")
                    nc.tensor.transpose(out=dnt_ps[:], in_=dn[:, t, :],
                                        identity=ident[:])
                    dnt = pa_pool.tile([P, P], F32, tag="dnts")
                    nc.vector.tensor_copy(out=dnt[:], in_=dnt_ps[:])
                    # out[d, ho] = sum_h dn[d,h] * We[ho,h]
                    mm_ps = pa_psum.tile([P, H], F32, tag="mm")
                    nc.tensor.matmul(out=mm_ps[:], lhsT=dnt[:],
                                     rhs=wet[:], start=True, stop=True)
                    z = pa_pool.tile([P, H], table_dt, tag="az")
                    nc.vector.tensor_copy(out=z[:], in_=mm_ps[:])
                    w = nc.sync.dma_start(
                        out=dwn[d0 + t * P:d0 + (t + 1) * P, :], in_=z[:])
                    table_writes.append(w)

            # ragged dis tail (rows 4992..4999)
            for _ in range(1 if phase_a else 0):
                rows = N_DIS - N_DIS_FULL                # 8
                d0 = N_DIS_FULL
                x = pa_pool.tile([P, ST, H], F32, tag="dx")
                nc.sync.dma_start(
                    out=x[:rows, 0, :],
                    in_=emb[N_RNA + d0:N_RNA + d0 + rows, :])
                rec = _norm_scale_wide(nc, pa_small, x[:rows, :1, :], rows, 1)
                dn = pa_pool.tile([P, ST, H], F32, tag="dn")
                nc.vector.tensor_tensor(
                    out=dn[:rows, :1, :], in0=x[:rows, :1, :],
                    in1=rec[:rows, :1].to_broadcast([rows, 1, H]),
                    op=mybir.AluOpType.mult)
                dnt_ps = pa_psum.tile([P, P], F32, tag="dnt")
                nc.tensor.transpose(out=dnt_ps[:, :rows], in_=dn[:rows, 0, :],
                                    identity=ident[:rows, :rows])
                dnt = pa_pool.tile([P, P], F32, tag="dnts")
                nc.vector.tensor_copy(out=dnt[:, :rows], in_=dnt_ps[:, :rows])
                mm_ps = pa_psum.tile([P, H], F32, tag="mm")
                nc.tensor.matmul(out=mm_ps[:rows, :], lhsT=dnt[:, :rows],
                                 rhs=wet[:], start=True, stop=True)
                z = pa_pool.tile([P, H], table_dt, tag="az")
                nc.vector.tensor_copy(out=z[:rows], in_=mm_ps[:rows, :])
                w = nc.sync.dma_start(out=dwn[d0:d0 + rows, :], in_=z[:rows])
                table_writes.append(w)

            # Fence: every gather must observe the completed tables.
            fence = nc.gpsimd.nop(nofuse=True, hint="table_fence")
            for w in table_writes:
                add_dep_helper(fence.ins, w.ins, reason="fence waits on table writes")

            # --- Phase B: gather + dot per 2048-edge block -----------------
            # dma_gather places row i at dest[p=i%128, slot=i//128], so edge
            # q (block-local) = s*128 + p lands at score[p, s].  A group of 8
            # blocks gives score_grp[128, 128] with col c = b_loc*16 + s; its
            # PE transpose has row c = 128 contiguous output edges.
            # Load both wrapped index planes once, replicated 8x down the
            # partition axis (one 16-partition copy per pair of Q7 cores).
            ridx_s = const_pool.tile([P, n_blocks * ipw], mybir.dt.int16)
            didx_s = const_pool.tile([P, n_blocks * ipw], mybir.dt.int16)
            for c in range(8):
                nc.sync.dma_start(out=ridx_s[16 * c:16 * (c + 1), :], in_=ridx[:, :])
                nc.sync.dma_start(out=didx_s[16 * c:16 * (c + 1), :], in_=didx[:, :])

            score = None
            for _rep_b in range((n_blocks if phase_b else 0) * repeat_b):
                b = _rep_b % max(n_blocks, 1)
                full = b < n_full
                n_valid = blk if full else tail

                g = b % group
                if g == 0:
                    score = pb_sc.tile([P, group * k], F32, tag="score")
                    if n_blocks - b < group:
                        # partial final group: zero unused columns so the
                        # full-tile transpose reads defined data
                        nc.vector.memset(score[:, :], 0.0)

                r = pb_gat.tile([P, k, H], table_dt, tag="r")
                d = pb_gat.tile([P, k, H], table_dt, tag="d")
                if not full:
                    # tail: gather skips the -1-padded rows; zero the tiles so
                    # the full-tile multiply/reduce reads defined data
                    nc.vector.memset(r[:, :, :], 0.0)
                    nc.vector.memset(d[:, :, :], 0.0)
                # dma_gather's Q7 ucode runs only on core pair `queue_num`
                # (cpu_id/2 == queue_num); rotating queues lets up to 4
                # gathers generate descriptors concurrently.
                q1 = (2 * b) % 4 if queue_rr else 0
                q2 = (2 * b + 1) % 4 if queue_rr else 0
                g1 = nc.gpsimd.dma_gather(
                    r[:, :, :], rna_n[:, :], ridx_s[:, b * ipw:(b + 1) * ipw],
                    num_idxs=blk, num_idxs_reg=n_valid,
                    elem_size=H, elem_step=H, single_packet=single_packet,
                    queue_num=q1)
                add_dep_helper(g1.ins, fence.ins, reason="gather after tables")
                if gathers >= 2:
                    g2 = nc.gpsimd.dma_gather(
                        d[:, :, :], dwn[:, :], didx_s[:, b * ipw:(b + 1) * ipw],
                        num_idxs=blk, num_idxs_reg=n_valid,
                        elem_size=H, elem_step=H, single_packet=single_packet,
                        queue_num=q2)
                    add_dep_helper(g2.ins, fence.ins, reason="gather after tables")
                elif compute:
                    nc.vector.memset(d[:, :, :], 0.5)

                if compute:
                    r2 = r[:, :, :].rearrange("p s h -> p (s h)")
                    d2 = d[:, :, :].rearrange("p s h -> p (s h)")
                    nc.vector.tensor_mul(r2, r2, d2)
                    nc.vector.reduce_sum(
                        out=score[:, g * k:(g + 1) * k], in_=r[:, :, :],
                        axis=mybir.AxisListType.X)

                # flush finished group via PE transpose -> contiguous DMA
                last_in_group = (g == group - 1) or (b == n_blocks - 1)
                if last_in_group and compute and flush:
                    e_g0 = (b - g) * blk
                    st_ps = pa_psum.tile([P, P], F32, tag="st")
                    nc.tensor.transpose(out=st_ps[:], in_=score[:, :],
                                        identity=ident[:])
                    st = pb_sc.tile([P, P], F32, tag="st_s")
                    nc.vector.tensor_copy(out=st[:], in_=st_ps[:])
                    n_out = (g * blk) + n_valid          # edges in this group
                    rows, rem = divmod(n_out, P)
                    if rows:
                        nc.sync.dma_start(
                            out=out[e_g0:e_g0 + rows * P].rearrange(
                                "(c p) -> c p", p=P),
                            in_=st[:rows, :])
                    if rem:
                        nc.sync.dma_start(
                            out=out[e_g0 + rows * P:e_g0 + n_out].rearrange(
                                "(o e) -> o e", o=1),
                            in_=st[rows:rows + 1, :rem])

    # Bacc pipeline: splits multi-waits into event semaphores (walrus allows
    # only one sync-wait per instruction), register alloc, DCE, etc.
    nc.compile()
    return nc


def build_program_v2():
    """Sort-by-dis design: the dis side of every edge dot product is
    reconstructed on the PE from an SBUF-resident dwn^T table via a one-hot
    matmul against a static 128-wide window, so only the rna side is an
    indirect gather (bf16, transpose mode, 4-way SWDGE queue round-robin).

    Per 128-edge tile t (edges host-sorted by dis, logits unpermuted there):
        G[e, j]  = rna_n[i_e] . dwn[w_t + j]      (PE: lhsT=r_T, rhs=dwn_T win)
        logit[e] = sum_j G[e, j] * (j == lj_e)    (DVE: is_equal, mult, reduce)
    """
    nc = bacc.Bacc(num_swdge_queues=4)

    emb = nc.dram_tensor("emb", [N_EMB, H], F32, kind="ExternalInput")
    we = nc.dram_tensor("We", [H, H], F32, kind="ExternalInput")
    ipw = BLK // 16
    ridx = nc.dram_tensor("rna_idx16", [16, N_BLK2 * ipw], mybir.dt.int16,
                          kind="ExternalInput")
    lj_in = nc.dram_tensor("lj16", [P, T_TILES], BF16, kind="ExternalInput")
    out = nc.dram_tensor("logits", [E_PAD], F32, kind="ExternalOutput")

    rna_n = nc.dram_tensor("rna_n", [N_RNA_PAD, H], BF16, kind="Internal")

    table_writes = []

    with tile.TileContext(nc) as tc:
        with (
            tc.tile_pool(name="const", bufs=1) as const_pool,
            tc.tile_pool(name="pa", bufs=3) as pa_pool,
            tc.tile_pool(name="pa_small", bufs=4) as pa_small,
            # PSUM tiles round up to 2KB banks: pa_psum 4 tags x 1 buf = 4
            # banks, st 1 bank, gp 3 banks -> 8 banks total.
            tc.tile_pool(name="pa_psum", bufs=1, space="PSUM") as pa_psum,
            tc.tile_pool(name="st_psum", bufs=1, space="PSUM") as st_psum,
            tc.tile_pool(name="gat", bufs=6) as gat_pool,
            tc.tile_pool(name="oh", bufs=4) as oh_pool,
            tc.tile_pool(name="pr", bufs=4) as pr_pool,
            tc.tile_pool(name="sc", bufs=2) as sc_pool,
            tc.tile_pool(name="gp", bufs=3, space="PSUM") as gp_pool,
        ):
            # --- We^T in SBUF (PE inputs routed through DVE copies) --------
            ident0 = const_pool.tile([P, P], F32)
            make_identity(nc, ident0[:])
            ident = const_pool.tile([P, P], F32)
            nc.vector.tensor_copy(out=ident[:], in_=ident0[:])
            we_s0 = const_pool.tile([P, H], F32)
            nc.sync.dma_start(out=we_s0[:], in_=we[:, :])
            we_s = const_pool.tile([P, H], F32)
            nc.vector.tensor_copy(out=we_s[:], in_=we_s0[:])
            wet_ps = pa_psum.tile([P, P], F32, tag="wet")
            nc.tensor.transpose(out=wet_ps[:], in_=we_s[:], identity=ident[:])
            wet = const_pool.tile([P, H], F32)   # wet[h, ho] = We[ho, h]
            nc.vector.tensor_copy(out=wet[:], in_=wet_ps[:])

            # --- A1: normalized rna table (bf16, DRAM) ---------------------
            n_st = (N_RNA_PAD // P + ST - 1) // ST
            for st in range(n_st):
                r0 = st * ST * P
                tw = min(ST, (N_RNA_PAD - r0) // P)
                x = pa_pool.tile([P, ST, H], F32, tag="ax")
                nc.sync.dma_start(
                    out=x[:, :tw, :],
                    in_=emb[r0:r0 + tw * P, :].rearrange("(t p) h -> p t h", p=P))
                rec = _norm_scale_wide(nc, pa_small, x[:, :tw, :], P, tw)
                y = pa_pool.tile([P, ST, H], BF16, tag="ay")
                nc.vector.tensor_tensor(
                    out=y[:, :tw, :], in0=x[:, :tw, :],
                    in1=rec[:, :tw].to_broadcast([P, tw, H]),
                    op=mybir.AluOpType.mult)
                w = nc.sync.dma_start(
                    out=rna_n[r0:r0 + tw * P, :].rearrange("(t p) h -> p t h", p=P),
                    in_=y[:, :tw, :])
                table_writes.append(w)

            # --- A2: dwn^T resident in SBUF [h, d] bf16 --------------------
            # dwn[d, ho] = sum_h We[ho, h] * dis_n[d, h]; transposed form
            # dwn_T[ho, d] = matmul(lhsT=wet[h, ho], rhs=dnt[h, d]).
            dwn_t = const_pool.tile([P, N_DIS], BF16)
            N_DIS_FULL = (N_DIS // P) * P               # 4992
            n_dst = (N_DIS_FULL // P + ST - 1) // ST
            for st in range(n_dst):
                d0 = st * ST * P
                tw = min(ST, (N_DIS_FULL - d0) // P)
                x = pa_pool.tile([P, ST, H], F32, tag="dx")
                nc.sync.dma_start(
                    out=x[:, :tw, :],
                    in_=emb[N_RNA + d0:N_RNA + d0 + tw * P, :].rearrange(
                        "(t p) h -> p t h", p=P))
                rec = _norm_scale_wide(nc, pa_small, x[:, :tw, :], P, tw)
                dn = pa_pool.tile([P, ST, H], F32, tag="dn")
                nc.vector.tensor_tensor(
                    out=dn[:, :tw, :], in0=x[:, :tw, :],
                    in1=rec[:, :tw].to_broadcast([P, tw, H]),
                    op=mybir.AluOpType.mult)
                for t in range(tw):
                    dnt_ps = pa_psum.tile([P, P], F32, tag="dnt")
                    nc.tensor.transpose(out=dnt_ps[:], in_=dn[:, t, :],
                                        identity=ident[:])
                    dnt = pa_pool.tile([P, P], F32, tag="dnts")
                    nc.vector.tensor_copy(out=dnt[:], in_=dnt_ps[:])
                    mm_ps = pa_psum.tile([P, P], F32, tag="mm")
                    nc.tensor.matmul(out=mm_ps[:], lhsT=wet[:],
                                     rhs=dnt[:], start=True, stop=True)
                    nc.vector.tensor_copy(
                        out=dwn_t[:, d0 + t * P:d0 + (t + 1) * P], in_=mm_ps[:])

            # dis tail rows 4992..4999
            rows = N_DIS - N_DIS_FULL                    # 8
            d0 = N_DIS_FULL
            x = pa_pool.tile([P, ST, H], F32, tag="dx")
            nc.sync.dma_start(
                out=x[:rows, 0, :], in_=emb[N_RNA + d0:N_RNA + d0 + rows, :])
            rec = _norm_scale_wide(nc, pa_small, x[:rows, :1, :], rows, 1)
            dn = pa_pool.tile([P, ST, H], F32, tag="dn")
            nc.vector.tensor_tensor(
                out=dn[:rows, :1, :], in0=x[:rows, :1, :],
                in1=rec[:rows, :1].to_broadcast([rows, 1, H]),
                op=mybir.AluOpType.mult)
            dnt_ps = pa_psum.tile([P, P], F32, tag="dnt")
            nc.tensor.transpose(out=dnt_ps[:, :rows], in_=dn[:rows, 0, :],
                                identity=ident[:rows, :rows])
            dnt = pa_pool.tile([P, P], F32, tag="dnts")
            nc.vector.tensor_copy(out=dnt[:, :rows], in_=dnt_ps[:, :rows])
            mm_ps = pa_psum.tile([P, P], F32, tag="mm")
            nc.tensor.matmul(out=mm_ps[:, :rows], lhsT=wet[:],
                             rhs=dnt[:, :rows], start=True, stop=True)
            nc.vector.tensor_copy(out=dwn_t[:, d0:d0 + rows],
                                  in_=mm_ps[:, :rows])

            # Fence: gathers must observe the completed rna_n table.
            fence = nc.gpsimd.nop(nofuse=True, hint="table_fence")
            for w in table_writes:
                add_dep_helper(fence.ins, w.ins, reason="fence waits on table writes")

            # --- Phase-B constants ----------------------------------------
            # wrapped rna index planes, one 16-partition copy per Q7 core
            ridx_s = const_pool.tile([P, N_BLK2 * ipw], mybir.dt.int16)
            for c in range(8):
                nc.sync.dma_start(out=ridx_s[16 * c:16 * (c + 1), :], in_=ridx[:, :])
            # window-local dis index per edge: lj_s[e, t]
            lj_s = const_pool.tile([P, T_TILES], BF16)
            nc.sync.dma_start(out=lj_s[:, :], in_=lj_in[:, :])
            # iota over the window axis, replicated TB wide
            io32 = const_pool.tile([P, TB, P], I32)
            nc.gpsimd.iota(io32[:, :, :], pattern=[[0, TB], [1, P]],
                           base=0, channel_multiplier=0)
            iob = const_pool.tile([P, TB, P], BF16)
            nc.vector.tensor_copy(out=iob[:, :, :], in_=io32[:, :, :])

            # --- Phase B: gather + windowed one-hot dot per tile -----------
            score = None
            for b in range(N_BLK2):
                rt = gat_pool.tile([P, 1, BLK], BF16, tag="rt")
                g = nc.gpsimd.dma_gather(
                    rt[:, :, :], rna_n[:, :], ridx_s[:, b * ipw:(b + 1) * ipw],
                    num_idxs=BLK, num_idxs_reg=BLK, elem_size=H,
                    transpose=True, queue_num=b % 4)
                add_dep_helper(g.ins, fence.ins, reason="gather after tables")

                for tb in range(BLK // P // TB):          # 4 batches of TB tiles
                    t0 = b * (BLK // P) + tb * TB
                    g_loc = t0 % SC_GROUP
                    if g_loc == 0:
                        score = sc_pool.tile([P, SC_GROUP], F32, tag="score")
                        if T_TILES - t0 < SC_GROUP:
                            nc.vector.memset(score[:, :], 0.0)
                    gp_t = gp_pool.tile([P, TB, P], F32, tag="gp")
                    for u in range(TB):
                        t = t0 + u
                        e0 = (tb * TB + u) * P
                        wt = V2_WINDOWS[t]
                        nc.tensor.matmul(
                            out=gp_t[:, u, :],
                            lhsT=rt[:, 0, e0:e0 + P],
                            rhs=dwn_t[:, wt:wt + P],
                            start=True, stop=True)
                    oh_t = oh_pool.tile([P, TB, P], BF16, tag="oh")
                    nc.vector.tensor_tensor(
                        out=oh_t[:, :, :], in0=iob[:, :, :],
                        in1=lj_s[:, t0:t0 + TB].to_broadcast([P, TB, P]),
                        op=mybir.AluOpType.is_equal)
                    pr_t = pr_pool.tile([P, TB, P], F32, tag="pr")
                    nc.vector.tensor_tensor(
                        out=pr_t[:, :, :], in0=gp_t[:, :, :], in1=oh_t[:, :, :],
                        op=mybir.AluOpType.mult)
                    nc.vector.reduce_sum(
                        out=score[:, g_loc:g_loc + TB], in_=pr_t[:, :, :],
                        axis=mybir.AxisListType.X)

                    # flush finished score group (128 tiles = 16384 edges)
                    t_next = t0 + TB
                    if t_next % SC_GROUP == 0 or t_next == T_TILES:
                        grp = t0 // SC_GROUP
                        n_rows = min(T_TILES - grp * SC_GROUP, SC_GROUP)
                        st_ps = st_psum.tile([P, P], F32, tag="st")
                        nc.tensor.transpose(out=st_ps[:], in_=score[:, :],
                                            identity=ident[:])
                        stt = sc_pool.tile([P, P], F32, tag="st_s")
                        nc.vector.tensor_copy(out=stt[:], in_=st_ps[:])
                        e_g0 = grp * SC_GROUP * P
                        nc.sync.dma_start(
                            out=out[e_g0:e_g0 + n_rows * P].rearrange(
                                "(c p) -> c p", p=P),
                            in_=stt[:n_rows, :])

    nc.compile()
    return nc


_PROGRAM_CACHE = {}


def _get_program(table_dt=TABLE_DT):
    key = str(table_dt)
    if key not in _PROGRAM_CACHE:
        _PROGRAM_CACHE[key] = build_program(table_dt)
    return _PROGRAM_CACHE[key]


def _get_program_v2():
    if "v2" not in _PROGRAM_CACHE:
        _PROGRAM_CACHE["v2"] = build_program_v2()
    return _PROGRAM_CACHE["v2"]


def wrap_indices(idx, k=K):
    """[E_CORE] int -> dma_gather wrapped plane [16, n_blocks*k] int16.

    Block b's blk indices (tail padded with -1) occupy plane columns
    [b*k, (b+1)*k) with element i at [i % 16, i // 16].
    """
    blk, n_full, tail, group, n_blocks = geom(k)
    padded = np.full(n_blocks * blk, -1, dtype=np.int16)
    padded[:len(idx)] = idx.astype(np.int16)
    blocks = padded.reshape(n_blocks, blk // 16, 16).transpose(0, 2, 1)
    return np.ascontiguousarray(
        blocks.transpose(1, 0, 2).reshape(16, n_blocks * (blk // 16)))


def _make_in_maps(emb, We, rna_all, dis_all, k=K):
    in_maps = []
    for c in range(N_CORES):
        sl = slice(c * E_CORE, (c + 1) * E_CORE)
        in_maps.append({
            "emb": np.ascontiguousarray(emb, dtype=np.float32),
            "We": np.ascontiguousarray(We, dtype=np.float32),
            "rna_idx16": wrap_indices(np.asarray(rna_all[sl]), k),
            "dis_idx16": wrap_indices(np.asarray(dis_all[sl]), k),
        })
    return in_maps


def wrap_indices_v2(idx):
    """[E_PAD] int -> wrapped plane [16, N_BLK2*128] int16 (pads use idx 0)."""
    blocks = idx.astype(np.int16).reshape(N_BLK2, BLK // 16, 16).transpose(0, 2, 1)
    return np.ascontiguousarray(
        blocks.transpose(1, 0, 2).reshape(16, N_BLK2 * (BLK // 16)))


def _prep_core_v2(rna_c, dis_c):
    """Sort a core's edges by dis; returns (ridx plane, lj plane, perm)."""
    import ml_dtypes
    perm = np.argsort(dis_c, kind="stable")
    rna_sorted = np.zeros(E_PAD, np.int64)
    rna_sorted[:E_CORE] = rna_c[perm]
    lj = np.full(E_PAD, LJ_SENTINEL, np.int64)
    dis_sorted = dis_c[perm]
    wins = np.asarray(V2_WINDOWS, np.int64)
    lj_real = dis_sorted - np.repeat(wins, P)[:E_CORE]
    if lj_real.min() < 0 or lj_real.max() > P - 1:
        raise AssertionError(
            f"v2 window overflow: lj range [{lj_real.min()}, {lj_real.max()}]")
    lj[:E_CORE] = lj_real
    lj_plane = np.ascontiguousarray(
        lj.reshape(T_TILES, P).T.astype(ml_dtypes.bfloat16))
    return wrap_indices_v2(rna_sorted), lj_plane, perm


def kernel_run_v2(emb, We, pos_rna, pos_dis, neg_rna, neg_dis, rna_num,
                  trace=False):
    """Returns ((logits, label), exec_time_ns_or_None)."""
    from concourse.bass_utils import run_bass_kernel_spmd

    emb = np.ascontiguousarray(np.asarray(emb), dtype=np.float32)
    We = np.ascontiguousarray(np.asarray(We), dtype=np.float32)
    rna_all = np.concatenate([np.asarray(pos_rna), np.asarray(neg_rna)])
    dis_all = np.concatenate([np.asarray(pos_dis), np.asarray(neg_dis)])
    assert emb.shape == (N_EMB, H) and We.shape == (H, H)
    assert rna_all.shape == (E_TOT,) and dis_all.shape == (E_TOT,)

    nc = _get_program_v2()
    in_maps, perms = [], []
    for c in range(N_CORES):
        sl = slice(c * E_CORE, (c + 1) * E_CORE)
        ridx, lj_plane, perm = _prep_core_v2(rna_all[sl], dis_all[sl])
        perms.append(perm)
        in_maps.append({"emb": emb, "We": We,
                        "rna_idx16": ridx, "lj16": lj_plane})
    res = run_bass_kernel_spmd(
        nc, in_maps, core_ids=list(range(N_CORES)), trace=trace)
    global _LAST_RES
    _LAST_RES = res

    logits = np.empty(E_TOT, np.float32)
    for c in range(N_CORES):
        sorted_logits = np.asarray(res.results[c]["logits"])[:E_CORE]
        logits[c * E_CORE + perms[c]] = sorted_logits
    n_pos = np.asarray(pos_rna).shape[0]
    n_neg = np.asarray(neg_rna).shape[0]
    label = np.concatenate([np.ones(n_pos, np.float32),
                            np.zeros(n_neg, np.float32)])
    return (logits, label), res.exec_time_ns


def kernel_run(emb, We, pos_rna, pos_dis, neg_rna, neg_dis, rna_num,
               trace=False, table_dt=TABLE_DT):
    """Returns ((logits, label), exec_time_ns_or_None)."""
    from concourse.bass_utils import run_bass_kernel_spmd

    emb = np.asarray(emb)
    We = np.asarray(We)
    rna_all = np.concatenate([np.asarray(pos_rna), np.asarray(neg_rna)])
    dis_all = np.concatenate([np.asarray(pos_dis), np.asarray(neg_dis)])
    assert emb.shape == (N_EMB, H) and We.shape == (H, H)
    assert rna_all.shape == (E_TOT,) and dis_all.shape == (E_TOT,)

    nc = _get_program(table_dt)
    in_maps = _make_in_maps(emb, We, rna_all, dis_all)
    res = run_bass_kernel_spmd(
        nc, in_maps, core_ids=list(range(N_CORES)), trace=trace)
    global _LAST_RES
    _LAST_RES = res

    logits = np.concatenate([res.results[c]["logits"] for c in range(N_CORES)])
    n_pos = np.asarray(pos_rna).shape[0]
    n_neg = np.asarray(neg_rna).shape[0]
    label = np.concatenate([np.ones(n_pos, np.float32),
                            np.zeros(n_neg, np.float32)])
    return (logits.astype(np.float32), label), res.exec_time_ns


USE_V2 = True


def kernel(**inputs):
    if USE_V2:
        (logits, label), _ = kernel_run_v2(**inputs)
    else:
        (logits, label), _ = kernel_run(**inputs)
    return (logits, label)

